# revision 36
# baseline (speedup 1.0000x reference)
"""DGCNN forward on 8 Trainium2 NeuronCores (Bass/Tile), data-parallel.

Math restructuring vs the reference (exact equivalences):
  * Edge-conv z_iq = U[:, idx(i,q)] + V[:, i], U = Wn X, V = (Wc-Wn) X.
  * BN+lrelu is monotone per channel (g > 0) so max_q commutes:
    x_out = lrelu(bn(max_q z)). BN stats over all N*K edges via
      S1 = deg.U + K 1.V ;  S2 = deg.U^2 + 2 sum_c U.(V^T A) + K 1.V^2
    with deg/cross from the 0/1 knn adjacency A (PE matmuls).
  * knn ranking via s_ij = x_i.x_j - |x_j|^2/2 (monotone transform of
    the reference's neg_dist per row).
  * Layer-3 max-pool commutes with bn+lrelu likewise (max_i z3 first).
  * Only BN statistics cross cores -> 3 small AllReduces.
"""
import os
import numpy as np

import concourse.bass as bass
import concourse.bacc as bacc
import concourse.mybir as mybir
import concourse.tile as tile
from concourse import library_config
from concourse.bass_utils import run_bass_kernel_spmd

F32 = mybir.dt.float32
F32R = mybir.dt.float32r
BF16 = mybir.dt.bfloat16
U16 = mybir.dt.uint16
I16 = mybir.dt.int16

NC = 8
S = 8
N = 512
K = 20
EPS = 1e-5
SLOPE = 0.2
NEG = -1e30

AOP = mybir.AluOpType
AFT = mybir.ActivationFunctionType
AXL = mybir.AxisListType

L2_BF16 = os.environ.get("KERNEL_L2F32") != "1"
PHASES = int(os.environ.get("KERNEL_PHASES", "6"))


def build_nc():
    nc = bacc.Bacc("TRN2", target_bir_lowering=False, debug=False, num_devices=NC)

    xs = nc.dram_tensor("xs", [S, N, 3], F32, kind="ExternalInput").ap()
    wnt1 = nc.dram_tensor("wnt1", [3, 64], F32, kind="ExternalInput").ap()
    wdt1 = nc.dram_tensor("wdt1", [3, 64], F32, kind="ExternalInput").ap()
    wnt2 = nc.dram_tensor("wnt2", [64, 256], F32, kind="ExternalInput").ap()
    wdt2 = nc.dram_tensor("wdt2", [64, 256], F32, kind="ExternalInput").ap()
    w3t = nc.dram_tensor("w3t", [320, 512], F32, kind="ExternalInput").ap()
    g1c = nc.dram_tensor("g1c", [64, 1], F32, kind="ExternalInput").ap()
    b1c = nc.dram_tensor("b1c", [64, 1], F32, kind="ExternalInput").ap()
    g2c = nc.dram_tensor("g2c", [128, 2], F32, kind="ExternalInput").ap()
    b2c = nc.dram_tensor("b2c", [128, 2], F32, kind="ExternalInput").ap()
    g3c = nc.dram_tensor("g3c", [128, 4], F32, kind="ExternalInput").ap()
    b3c = nc.dram_tensor("b3c", [128, 4], F32, kind="ExternalInput").ap()
    eye = nc.dram_tensor("eye", [128, 128], F32, kind="ExternalInput").ap()
    out = nc.dram_tensor("out", [S, 1024], F32, kind="ExternalOutput").ap()
    dbg1 = nc.dram_tensor("dbg1", [64, 512], F32, kind="ExternalOutput").ap()
    dbg2 = nc.dram_tensor("dbg2", [128, 512], F32, kind="ExternalOutput").ap()
    dbg3 = nc.dram_tensor("dbg3", [128, 8], F32, kind="ExternalOutput").ap()

    n1 = float(NC * S * N * K)
    n3 = float(NC * S * N)

    tcctx = tile.TileContext(nc)
    tc = tcctx.__enter__()
    if True:

        cp = tc.alloc_tile_pool(name="consts", bufs=1)
        dramp = tc.alloc_tile_pool(name="dram", bufs=2, space="DRAM")
        dram1 = tc.alloc_tile_pool(name="dram1", bufs=1, space="DRAM")

        eye_s = cp.tile([128, 128], F32, tag="eye")
        nc.sync.dma_start(eye_s[:], eye[:])
        eye_b = cp.tile([128, 128], BF16, tag="eyeb")
        nc.vector.tensor_copy(eye_b[:], eye_s[:])
        w1n_s = cp.tile([3, 64], F32, tag="w1n"); nc.sync.dma_start(w1n_s[:], wnt1[:])
        w1d_s = cp.tile([3, 64], F32, tag="w1d"); nc.sync.dma_start(w1d_s[:], wdt1[:])
        w2n_s = cp.tile([64, 256], F32, tag="w2n"); nc.sync.dma_start(w2n_s[:], wnt2[:])
        w2d_s = cp.tile([64, 256], F32, tag="w2d"); nc.sync.dma_start(w2d_s[:], wdt2[:])
        w3ts = []
        for kc, (p0, pn) in enumerate(((0, 64), (64, 128), (192, 128))):
            t = cp.tile([pn, 512], F32, tag=f"w3t{kc}")
            nc.sync.dma_start(t[:], w3t[p0:p0 + pn, :])
            w3ts.append(t)
        g1_s = cp.tile([64, 1], F32, tag="g1"); nc.sync.dma_start(g1_s[:], g1c[:])
        b1_s = cp.tile([64, 1], F32, tag="b1"); nc.sync.dma_start(b1_s[:], b1c[:])
        g2_s = cp.tile([128, 2], F32, tag="g2"); nc.sync.dma_start(g2_s[:], g2c[:])
        b2_s = cp.tile([128, 2], F32, tag="b2"); nc.sync.dma_start(b2_s[:], b2c[:])
        g3_s = cp.tile([128, 4], F32, tag="g3"); nc.sync.dma_start(g3_s[:], g3c[:])
        b3_s = cp.tile([128, 4], F32, tag="b3"); nc.sync.dma_start(b3_s[:], b3c[:])
        neghalf = cp.tile([64, 1], F32, tag="neghalf")
        nc.vector.memset(neghalf[:], -0.5)
        onescol = cp.tile([128, 1], F32, tag="onescol")
        nc.vector.memset(onescol[:], 1.0)
        onesrow = cp.tile([1, 512], F32, tag="onesrow")
        nc.vector.memset(onesrow[:], 1.0)


        # persistent per-sample pools
        xb2p = tc.alloc_tile_pool(name="xb2", bufs=S)
        zmt1p = tc.alloc_tile_pool(name="zmt1", bufs=S)
        zmt2p = tc.alloc_tile_pool(name="zmt2", bufs=S)
        zm3p = tc.alloc_tile_pool(name="zm3", bufs=S)
        sacc = tc.alloc_tile_pool(name="sacc", bufs=1)

        xb1p = tc.alloc_tile_pool(name="xb1", bufs=2)
        wk = tc.alloc_tile_pool(name="wk", bufs=1)
        swp = tc.alloc_tile_pool(name="swp", bufs=2)
        afp = tc.alloc_tile_pool(name="afp", bufs=4)
        idxp = tc.alloc_tile_pool(name="idxp", bufs=4)
        gp = tc.alloc_tile_pool(name="gpool", bufs=2)
        usqp = tc.alloc_tile_pool(name="usqp", bufs=1)
        x2p = tc.alloc_tile_pool(name="x2p", bufs=2)
        smallp = tc.alloc_tile_pool(name="small", bufs=4)
        rowp = tc.alloc_tile_pool(name="rowp", bufs=1)
        gvsp = tc.alloc_tile_pool(name="gvsp", bufs=1)

        ps_s = tc.alloc_tile_pool(name="ps_s", bufs=1, space="PSUM")
        ps_uv = tc.alloc_tile_pool(name="ps_uv", bufs=1, space="PSUM")
        ps_gvu = tc.alloc_tile_pool(name="ps_gvu", bufs=2, space="PSUM")
        ps_misc = tc.alloc_tile_pool(name="ps_misc", bufs=1, space="PSUM")
        ps_stat = tc.alloc_tile_pool(name="ps_stat", bufs=1, space="PSUM")

        cr1 = sacc.tile([128, 1], F32, tag="cr1"); nc.vector.memset(cr1[:], 0.0)
        cr2 = sacc.tile([128, 2], F32, tag="cr2"); nc.vector.memset(cr2[:], 0.0)
        s3sq = sacc.tile([128, 4], F32, tag="s3sq"); nc.vector.memset(s3sq[:], 0.0)
        s3s1 = sacc.tile([128, 4], F32, tag="s3s1"); nc.vector.memset(s3s1[:], 0.0)
        acc1 = sacc.tile([2, 256], F32, tag="acc1"); nc.vector.memset(acc1[:], 0.0)
        acc2 = sacc.tile([66, 512], F32, tag="acc2"); nc.vector.memset(acc2[:], 0.0)

        stA = ps_stat.tile([128, 512], F32, tag="stA")
        st3 = ps_stat.tile([128, 4], F32, tag="st3")

        L2G = BF16 if L2_BF16 else F32
        LCFG = {
            1: dict(C=3, Co=64, ncc=1, gdt=F32, zdt=F32, zp=zmt1p, cr=cr1),
            2: dict(C=64, Co=256, ncc=2, gdt=L2G, zdt=L2G, zp=zmt2p, cr=cr2),
        }

        def front(s_i, layer, xb):
            cfg = LCFG[layer]
            C, Co, ncc = cfg["C"], cfg["Co"], cfg["ncc"]
            gdt = cfg["gdt"]
            wn, wd = (w1n_s, w1d_s) if layer == 1 else (w2n_s, w2d_s)
            first = (s_i == 0)
            last = (s_i == S - 1)

            # xx row; XR = [X; -xx/2]
            x2sq = wk.tile([64, 512], F32, tag="x2sq")
            nc.scalar.activation(x2sq[0:C, :], xb[0:C, :], AFT.Square)
            xxp = ps_uv.tile([1, 512], F32, tag="uv")
            nc.tensor.matmul(xxp[:], neghalf[0:C, :], x2sq[0:C, :], start=True, stop=True)
            xr = wk.tile([65, 512], F32, tag="xr")
            nc.scalar.copy(xr[0:C, :], xb[0:C, :])
            xxs = wk.tile([1, 512], F32, tag="xxs")
            nc.scalar.copy(xxs[:], xxp[:])
            nc.sync.dma_start(xr[C:C + 1, :], xxs[:])

            # s-matrix chunks + topk scan
            af = []
            idx24 = []
            for h in range(2):
                sp = ps_s.tile([128, 1024], F32, tag="spsum")
                for q in range(2):
                    mc = h * 2 + q
                    nc.tensor.matmul(
                        sp[:, q * 512:(q + 1) * 512],
                        xb[0:C + 1, mc * 128:(mc + 1) * 128],
                        xr[0:C + 1, :], start=True, stop=True)
                sw2 = swp.tile([128, 1024], F32, tag="sw")
                nc.scalar.copy(sw2[:], sp[:])
                for q in range(2):
                    sw = sw2[:, q * 512:(q + 1) * 512]
                    i24 = idxp.tile([128, 24], U16, tag="idx24")
                    m8 = smallp.tile([128, 24], F32, tag="m8")
                    for r in range(3):
                        nc.vector.max(m8[:, 8 * r:8 * r + 8], sw)
                        nc.vector.max_index(i24[:, 8 * r:8 * r + 8],
                                            m8[:, 8 * r:8 * r + 8], sw)
                        if r < 2:
                            nc.vector.match_replace(sw, m8[:, 8 * r:8 * r + 8], sw, NEG)
                    rep8 = smallp.tile([128, 8], F32, tag="rep8")
                    nc.vector.tensor_copy(rep8[:, 0:4], m8[:, 16:20])
                    nc.vector.memset(rep8[:, 4:8], NEG)
                    nc.vector.match_replace(sw, rep8[:], sw, NEG)
                    a = afp.tile([128, 512], F32, tag="af")
                    nc.vector.tensor_scalar(a[:], sw, NEG, None, AOP.is_equal)
                    af.append(a)
                    idx24.append(i24)

            # int32 index tiles; U^T rows to DRAM; 80 indirect row-gathers
            idx32 = []
            for mc in range(4):
                i32 = idxp.tile([128, 24], mybir.dt.int32, tag="idx32")
                nc.vector.tensor_copy(i32[:, 0:20], idx24[mc][:, 0:20])
                idx32.append(i32)

            usq = usqp.tile([128, 4, 4, Co], F32, tag="usq")
            utd = dramp.tile([N, Co], F32, tag=f"utd{layer}")
            for ic in range(4):
                uvp = ps_uv.tile([128, 2 * Co], F32, tag="uv")
                nc.tensor.matmul(uvp[:, 0:Co], xb[0:C, ic * 128:(ic + 1) * 128],
                                 wn[:], start=True, stop=True)
                nc.tensor.matmul(uvp[:, Co:2 * Co], xb[0:C, ic * 128:(ic + 1) * 128],
                                 wd[:], start=True, stop=True)
                nc.scalar.copy(usq[:, ic, 0:2, :], uvp[:])
                nc.scalar.activation(usq[:, ic, 2:4, :], uvp[:], AFT.Square)
                nc.sync.dma_start(utd[ic * 128:(ic + 1) * 128, :], usq[:, ic, 0, :])

            zmt = cfg["zp"].tile([128, 4, Co], F32, tag=f"zm{layer}")
            for ic in range(4):
                g = gp.tile([128, K, Co], F32, tag="g")
                for q in range(K):
                    nc.gpsimd.indirect_dma_start(
                        out=g[:, q, :], out_offset=None,
                        in_=utd[:],
                        in_offset=bass.IndirectOffsetOnAxis(
                            ap=idx32[ic][:, q:q + 1], axis=0))
                for (na, nb) in ((10, 10), (5, 5), (2, 3), (1, 2), (1, 1)):
                    nc.vector.tensor_tensor(g[:, 0:nb, :], g[:, 0:nb, :],
                                            g[:, na:na + nb, :], AOP.max)
                nc.vector.tensor_tensor(zmt[:, ic, :], g[:, 0, :],
                                        usq[:, ic, 1, :], AOP.add)

            # cross term
            for cc in range(ncc):
                pn = min(128, Co)
                up = ps_gvu.tile([128, 512], F32, tag="gvu")
                nc.tensor.matmul(up[0:pn, :], wn[:, cc * 128:cc * 128 + pn],
                                 xb[0:C, :], start=True, stop=True)
                us = gvsp.tile([128, 512], F32, tag="us")
                nc.scalar.copy(us[0:pn, :], up[0:pn, :])
                gvp = ps_gvu.tile([128, 512], F32, tag="gvu")
                for ic in range(4):
                    nc.tensor.matmul(gvp[0:pn, :],
                                     usq[:, ic, 1, cc * 128:cc * 128 + pn],
                                     af[ic][:], start=(ic == 0), stop=(ic == 3),
                                     skip_group_check=True)
                gvs = gvsp.tile([128, 512], F32, tag="gvs")
                nc.scalar.copy(gvs[0:pn, :], gvp[0:pn, :])
                crx = smallp.tile([128, 1], F32, tag="crx")
                nc.vector.scalar_tensor_tensor(gvs[0:pn, :], us[0:pn, :], 1.0,
                                               gvs[0:pn, :], AOP.mult, AOP.mult,
                                               accum_out=crx[0:pn, :])
                cr = cfg["cr"]
                nc.vector.tensor_tensor(cr[0:pn, cc:cc + 1], cr[0:pn, cc:cc + 1],
                                        crx[0:pn, :], AOP.add)

            # ---- stats ----
            degc = smallp.tile([128, 4], F32, tag="degc")
            for jc in range(4):
                dp = ps_misc.tile([128, 512], F32, tag="misc")
                for ic in range(4):
                    nc.tensor.matmul(dp[:, 0:1], af[ic][:, jc * 128:(jc + 1) * 128],
                                     onescol[:], start=(ic == 0), stop=(ic == 3),
                                     skip_group_check=True)
                nc.scalar.copy(degc[:, jc:jc + 1], dp[:, 0:1])

            # matvec stats into persistent PSUM rows
            for ic in range(4):
                lhs = smallp.tile([128, 2], F32, tag="lhs2")
                nc.vector.tensor_copy(lhs[:, 0:1], degc[:, ic:ic + 1])
                nc.vector.tensor_copy(lhs[:, 1:2], onescol[:])
                if layer == 1:
                    nc.tensor.matmul(
                        stA[0:2, 0:256], lhs[:],
                        usq[:, ic, :, :].rearrange("p a co -> p (a co)"),
                        start=(ic == 0), stop=(ic == 3),
                        skip_group_check=True)
                else:
                    nc.tensor.matmul(
                        stA[32:34, :], lhs[:],
                        usq[:, ic, 0:2, :].rearrange("p a co -> p (a co)"),
                        start=(ic == 0), stop=(ic == 3),
                        skip_group_check=True)
                    nc.tensor.matmul(
                        stA[64:66, :], lhs[:],
                        usq[:, ic, 2:4, :].rearrange("p a co -> p (a co)"),
                        start=(ic == 0), stop=(ic == 3),
                        skip_group_check=True)
            if layer == 1:
                srow = rowp.tile([2, 512], F32, tag="srow")
                nc.scalar.copy(srow[0:2, 0:256], stA[0:2, 0:256])
                nc.vector.tensor_tensor(acc1[:], acc1[:], srow[0:2, 0:256], AOP.add)
            else:
                srow = rowp.tile([66, 512], F32, tag="srow66")
                nc.scalar.copy(srow[32:34, :], stA[32:34, :])
                nc.scalar.copy(srow[64:66, :], stA[64:66, :])
                nc.vector.tensor_tensor(acc2[32:34, :], acc2[32:34, :], srow[32:34, :], AOP.add)
                nc.vector.tensor_tensor(acc2[64:66, :], acc2[64:66, :], srow[64:66, :], AOP.add)

            return zmt

        def stats_finalize(layer, gcol, bcol):
            cfg = LCFG[layer]
            Co, ncc = cfg["Co"], cfg["ncc"]
            pn = min(128, Co)
            if layer == 1:
                sl = dict(s1a=acc1[0:1, 0:64], s1b=acc1[1:2, 64:128],
                          s2a=acc1[0:1, 128:192], s2b=acc1[1:2, 192:256])
            else:
                sl = dict(s1a=acc2[32:33, 0:256], s1b=acc2[33:34, 256:512],
                          s2a=acc2[64:65, 0:256], s2b=acc2[65:66, 256:512])
            nb = 5 * Co
            bin_ = dram1.tile([nb], F32, tag=f"arin{layer}")
            bout = dram1.tile([nb], F32, tag=f"arout{layer}")
            nc.sync.dma_start(bin_[0:Co].rearrange("(o c) -> o c", o=1), sl["s1a"])
            nc.sync.dma_start(bin_[Co:2 * Co].rearrange("(o c) -> o c", o=1), sl["s1b"])
            nc.sync.dma_start(bin_[2 * Co:3 * Co].rearrange("(o c) -> o c", o=1), sl["s2a"])
            nc.sync.dma_start(bin_[3 * Co:4 * Co].rearrange("(o c) -> o c", o=1), sl["s2b"])
            nc.sync.dma_start(
                bin_[4 * Co:5 * Co].rearrange("(cc p) -> p cc", p=pn),
                cfg["cr"][0:pn, 0:ncc])
            nc.gpsimd.collective_compute(
                "AllReduce", AOP.add, replica_groups=[list(range(NC))],
                ins=[bin_.opt()], outs=[bout.opt()])
            r1a = rowp.tile([1, 256], F32, tag="r1a")
            r1b = rowp.tile([1, 256], F32, tag="r1b")
            r2a = rowp.tile([1, 256], F32, tag="r2a")
            r2b = rowp.tile([1, 256], F32, tag="r2b")
            rcr = rowp.tile([1, 256], F32, tag="rcr")
            for t, o in ((r1a, 0), (r1b, 1), (r2a, 2), (r2b, 3), (rcr, 4)):
                nc.sync.dma_start(t[0:1, 0:Co],
                                  bout[o * Co:(o + 1) * Co].rearrange("(o c) -> o c", o=1))
            mean = rowp.tile([1, 256], F32, tag="meanr")
            nc.vector.scalar_tensor_tensor(mean[0:1, 0:Co], r1b[0:1, 0:Co], float(K),
                                           r1a[0:1, 0:Co], AOP.mult, AOP.add)
            nc.vector.tensor_scalar(mean[0:1, 0:Co], mean[0:1, 0:Co], 1.0 / n1,
                                    None, AOP.mult)
            ex2 = rowp.tile([1, 256], F32, tag="ex2r")
            nc.vector.scalar_tensor_tensor(ex2[0:1, 0:Co], rcr[0:1, 0:Co], 2.0,
                                           r2a[0:1, 0:Co], AOP.mult, AOP.add)
            nc.vector.scalar_tensor_tensor(ex2[0:1, 0:Co], r2b[0:1, 0:Co], float(K),
                                           ex2[0:1, 0:Co], AOP.mult, AOP.add)
            nc.vector.tensor_scalar(ex2[0:1, 0:Co], ex2[0:1, 0:Co], 1.0 / n1,
                                    None, AOP.mult)
            var = rowp.tile([1, 256], F32, tag="varr")
            nc.vector.scalar_tensor_tensor(var[0:1, 0:Co], mean[0:1, 0:Co], 1.0,
                                           mean[0:1, 0:Co], AOP.mult, AOP.mult)
            nc.vector.tensor_tensor(var[0:1, 0:Co], ex2[0:1, 0:Co], var[0:1, 0:Co],
                                    AOP.subtract)
            nc.vector.tensor_scalar(var[0:1, 0:Co], var[0:1, 0:Co], EPS, None, AOP.add)
            rec = rowp.tile([1, 256], F32, tag="recr")
            nc.vector.reciprocal(rec[0:1, 0:Co], var[0:1, 0:Co])
            rsq = rowp.tile([1, 256], F32, tag="rsqr")
            nc.scalar.activation(rsq[0:1, 0:Co], rec[0:1, 0:Co], AFT.Sqrt)
            sb = dram1.tile([2 * Co], F32, tag=f"sb{layer}")
            nc.sync.dma_start(sb[0:Co].rearrange("(o c) -> o c", o=1), rsq[0:1, 0:Co])
            nc.sync.dma_start(sb[Co:2 * Co].rearrange("(o c) -> o c", o=1), mean[0:1, 0:Co])
            rsqc = smallp.tile([128, 2], F32, tag=f"rsqc{layer}")
            meanc = smallp.tile([128, 2], F32, tag=f"meanc{layer}")
            nc.sync.dma_start(rsqc[0:pn, 0:ncc],
                              sb[0:Co].rearrange("(cc p) -> p cc", p=pn))
            nc.sync.dma_start(meanc[0:pn, 0:ncc],
                              sb[Co:2 * Co].rearrange("(cc p) -> p cc", p=pn))
            scc = sacc.tile([128, 2], F32, tag=f"scc{layer}")
            shc = sacc.tile([128, 2], F32, tag=f"shc{layer}")
            nc.vector.tensor_tensor(scc[0:pn, 0:ncc], gcol[0:pn, 0:ncc],
                                    rsqc[0:pn, 0:ncc], AOP.mult)
            nc.vector.tensor_tensor(shc[0:pn, 0:ncc], meanc[0:pn, 0:ncc],
                                    scc[0:pn, 0:ncc], AOP.mult)
            nc.vector.tensor_tensor(shc[0:pn, 0:ncc], bcol[0:pn, 0:ncc],
                                    shc[0:pn, 0:ncc], AOP.subtract)
            return scc, shc


        def emit_lrelu(dst, src, sc_ap, sh_ap, pn, accum=None):
            """dst = lrelu(src*sc + sh), slope 0.2; optional accum_out on final op."""
            tt = wk.tile([128, 512], F32, tag="lr_t")
            rr = wk.tile([128, 512], F32, tag="lr_r")
            nc.scalar.activation(tt[0:pn, :], src, AFT.Identity, bias=sh_ap, scale=sc_ap)
            nc.scalar.activation(rr[0:pn, :], src, AFT.Relu, bias=sh_ap, scale=sc_ap)
            nc.vector.scalar_tensor_tensor(rr[0:pn, :], rr[0:pn, :], 4.0, tt[0:pn, :],
                                           AOP.mult, AOP.add)
            if accum is None:
                nc.vector.tensor_scalar(dst, rr[0:pn, :], 0.2, None, AOP.mult)
            else:
                nc.vector.tensor_scalar(dst, rr[0:pn, :], 0.2, 0.0, AOP.mult,
                                        AOP.add, accum_out=accum)

        # ===== phase 1 =====
        zm1 = []
        for s_i in range(S):
            xb = xb1p.tile([4, 512], F32, tag="xb1")
            nc.sync.dma_start(xb[0:3, :], xs[s_i].rearrange("n d -> d n"))
            nc.sync.dma_start(xb[3:4, :], onesrow[:])
            zm1.append(front(s_i, 1, xb))

        if PHASES >= 2:
            sc1, sh1 = stats_finalize(1, g1_s, b1_s)

        # ===== phase 2 =====
        zm2 = []
        xb2s = []
        for s_i in range(S if PHASES >= 3 else 0):
            px = ps_misc.tile([128, 512], F32, tag="misc")
            for ic in range(4):
                nc.tensor.matmul(px[0:64, ic * 128:(ic + 1) * 128], zm1[s_i][:, ic, :],
                                 eye_s[:], is_transpose=True, start=True, stop=True)
            xb2 = xb2p.tile([65, 512], F32, tag="xb2")
            emit_lrelu(xb2[0:64, :], px[0:64, :], sc1[0:64, 0:1], sh1[0:64, 0:1], 64)
            nc.sync.dma_start(xb2[64:65, :], onesrow[:])
            if s_i == 0:
                nc.sync.dma_start(dbg3[0:64, 0:1], sc1[0:64, 0:1])
                nc.sync.dma_start(dbg3[0:64, 1:2], sh1[0:64, 0:1])
            xb2s.append(xb2)
            zm2.append(front(s_i, 2, xb2))

        if PHASES >= 4:
            sc2, sh2 = stats_finalize(2, g2_s, b2_s)

        def build_x2(s_i, cc):
            px = ps_misc.tile([128, 512], F32, tag="misc")
            for ic in range(4):
                nc.tensor.matmul(px[:, ic * 128:(ic + 1) * 128],
                                 zm2[s_i][:, ic, cc * 128:(cc + 1) * 128],
                                 eye_s[:], is_transpose=True, start=True, stop=True)
            x2 = x2p.tile([128, 512], F32, tag="x2")
            return px[:], x2

        # ===== phase 3 =====
        zm3s = []
        for s_i in range(S if PHASES >= 5 else 0):
            catsum = smallp.tile([128, 3], F32, tag="catsum")
            x2ab = []
            for cc in range(2):
                px, x2 = build_x2(s_i, cc)
                emit_lrelu(x2[:], px[:], sc2[:, cc:cc + 1], sh2[:, cc:cc + 1], 128,
                           accum=catsum[:, 1 + cc:2 + cc])
                if s_i == 0 and cc == 0:
                    nc.sync.dma_start(dbg2[:], x2[:])
                x2ab.append(x2)
            x1scr = wk.tile([64, 512], F32, tag="x1scr")
            nc.scalar.activation(x1scr[:], xb2s[s_i][0:64, :], AFT.Copy,
                                 accum_out=catsum[0:64, 0:1])
            csl = [(catsum[0:64, 0:1], 0), (catsum[0:128, 1:2], 1),
                   (catsum[0:128, 2:3], 2)]
            for cc in range(4):
                for kci, (cs, wi) in enumerate(csl):
                    nc.tensor.matmul(st3[:, cc:cc + 1],
                                     w3ts[wi][:, cc * 128:(cc + 1) * 128], cs,
                                     start=(kci == 0), stop=(kci == 2),
                                     skip_group_check=True)
            st3row = smallp.tile([128, 4], F32, tag="st3row")
            nc.scalar.copy(st3row[:], st3[:])
            nc.vector.tensor_tensor(s3s1[:], s3s1[:], st3row[:], AOP.add)
            zm3 = zm3p.tile([128, 4], F32, tag="zm3")
            rhss = [xb2s[s_i][0:64, :], x2ab[0][:], x2ab[1][:]]
            for cc in range(4):
                zp = ps_s.tile([128, 1024], F32, tag="spsum")
                for kci, rhs in enumerate(rhss):
                    nc.tensor.matmul(zp[:, 0:512],
                                     w3ts[kci][:, cc * 128:(cc + 1) * 128],
                                     rhs,
                                     start=(kci == 0), stop=(kci == 2))
                nc.vector.tensor_reduce(zm3[:, cc:cc + 1], zp[:, 0:512], AXL.X, AOP.max)
                sqs = wk.tile([128, 512], F32, tag="sqscr")
                sqa = smallp.tile([128, 1], F32, tag="sqa")
                nc.scalar.activation(sqs[:], zp[:, 0:512], AFT.Square, accum_out=sqa[:])
                nc.vector.tensor_tensor(s3sq[:, cc:cc + 1], s3sq[:, cc:cc + 1],
                                        sqa[:], AOP.add)
            zm3s.append(zm3)

        # stats3 AR
        if PHASES < 5:
            for pool in (ps_stat, ps_misc, ps_gvu, ps_uv, ps_s, gvsp, rowp, smallp,
                         x2p, usqp, gp, idxp, afp, swp, wk, xb1p, sacc, zm3p, zmt2p,
                         zmt1p, xb2p, dram1, dramp, cp):
                pool.release()
            tcctx.__exit__(None, None, None)
            nc.compile()
            return nc
        s3rows = rowp.tile([128, 8], F32, tag="s3rows")
        nc.vector.tensor_copy(s3rows[:, 0:4], s3s1[:])
        nc.vector.tensor_copy(s3rows[:, 4:8], s3sq[:])
        b3i = dram1.tile([1024], F32, tag="ar3in")
        b3o = dram1.tile([1024], F32, tag="ar3out")
        nc.sync.dma_start(b3i[:].rearrange("(p w) -> p w", p=128), s3rows[:])
        nc.gpsimd.collective_compute("AllReduce", AOP.add,
                                     replica_groups=[list(range(NC))],
                                     ins=[b3i.opt()], outs=[b3o.opt()])
        s3r = rowp.tile([128, 8], F32, tag="s3r")
        nc.sync.dma_start(s3r[:], b3o[:].rearrange("(p w) -> p w", p=128))
        mean3 = sacc.tile([128, 4], F32, tag="mean3")
        nc.vector.tensor_scalar(mean3[:], s3r[:, 0:4], 1.0 / n3, None, AOP.mult)
        var3 = sacc.tile([128, 4], F32, tag="var3")
        nc.vector.tensor_scalar(var3[:], s3r[:, 4:8], 1.0 / n3, None, AOP.mult)
        m3sq = smallp.tile([128, 4], F32, tag="m3sq")
        nc.vector.tensor_tensor(m3sq[:], mean3[:], mean3[:], AOP.mult)
        nc.vector.tensor_tensor(var3[:], var3[:], m3sq[:], AOP.subtract)
        nc.vector.tensor_scalar(var3[:], var3[:], EPS, None, AOP.add)
        rec3 = sacc.tile([128, 4], F32, tag="rec3")
        nc.vector.reciprocal(rec3[:], var3[:])
        sc3 = sacc.tile([128, 4], F32, tag="sc3")
        nc.scalar.activation(sc3[:], rec3[:], AFT.Sqrt)
        nc.vector.tensor_tensor(sc3[:], sc3[:], g3_s[:], AOP.mult)
        sh3 = sacc.tile([128, 4], F32, tag="sh3")
        nc.vector.tensor_tensor(sh3[:], mean3[:], sc3[:], AOP.mult)
        nc.vector.tensor_tensor(sh3[:], b3_s[:], sh3[:], AOP.subtract)
        nc.sync.dma_start(dbg3[:, 2:3], sc3[:, 0:1])
        nc.sync.dma_start(dbg3[:, 3:4], sh3[:, 0:1])
        nc.sync.dma_start(dbg3[:, 4:5], mean3[:, 0:1])
        nc.sync.dma_start(dbg3[:, 5:6], var3[:, 0:1])
        nc.sync.dma_start(dbg1[0:64, 0:4].rearrange("p q -> p q") if False else dbg1[0:1, 0:4], s3r[0:1, 0:4])

        # ===== phase 4 =====
        for s_i in range(S):
            x2ab = []
            for cc in range(2):
                px, x2 = build_x2(s_i, cc)
                emit_lrelu(x2[:], px[:], sc2[:, cc:cc + 1], sh2[:, cc:cc + 1], 128)
                x2ab.append(x2)
            rhss = [xb2s[s_i][0:64, :], x2ab[0][:], x2ab[1][:]]
            hsum = smallp.tile([128, 4], F32, tag="hsum")
            for cc in range(4):
                zp = ps_s.tile([128, 1024], F32, tag="spsum")
                for kci, rhs in enumerate(rhss):
                    nc.tensor.matmul(zp[:, 0:512],
                                     w3ts[kci][:, cc * 128:(cc + 1) * 128],
                                     rhs,
                                     start=(kci == 0), stop=(kci == 2))
                hscr = wk.tile([128, 512], F32, tag="hscr")
                emit_lrelu(hscr[:], zp[:, 0:512], sc3[:, cc:cc + 1], sh3[:, cc:cc + 1],
                           128, accum=hsum[:, cc:cc + 1])
            # pooled max via commuted bn+lrelu on zm3
            t = smallp.tile([128, 4], F32, tag="tmx")
            nc.vector.tensor_tensor(t[:], zm3s[s_i][:], sc3[:], AOP.mult)
            nc.vector.tensor_tensor(t[:], t[:], sh3[:], AOP.add)
            u = smallp.tile([128, 4], F32, tag="umx")
            nc.vector.tensor_scalar(u[:], t[:], SLOPE, None, AOP.mult)
            hmax = smallp.tile([128, 4], F32, tag="hmax")
            nc.vector.tensor_tensor(hmax[:], t[:], u[:], AOP.max)
            hmean = smallp.tile([128, 4], F32, tag="hmean")
            nc.vector.tensor_scalar(hmean[:], hsum[:], 1.0 / N, None, AOP.mult)
            nc.sync.dma_start(out[s_i, 0:512].rearrange("(cc p) -> p cc", p=128),
                              hmax[:])
            nc.sync.dma_start(out[s_i, 512:1024].rearrange("(cc p) -> p cc", p=128),
                              hmean[:])

        for pool in (ps_stat, ps_misc, ps_gvu, ps_uv, ps_s, gvsp, rowp, smallp,
                     x2p, usqp, gp, idxp, afp, swp, wk, xb1p, sacc, zm3p, zmt2p,
                     zmt1p, xb2p, dram1, dramp, cp):
            pool.release()
    tcctx.__exit__(None, None, None)

    nc.compile()
    return nc


_cached = {}


def kernel(x, w1, g1, b1, w2, g2, b2, w3, g3, b3):
    x = np.asarray(x, dtype=np.float32)
    B, P, Np, D = x.shape
    xs_all = x.reshape(B * P, Np, D)
    w1 = np.asarray(w1, np.float32)
    w2 = np.asarray(w2, np.float32)
    w3 = np.asarray(w3, np.float32)

    if "nc" not in _cached:
        _cached["nc"] = build_nc()
    nc = _cached["nc"]

    common = {
        "wnt1": np.ascontiguousarray(w1[:, 0:3].T),
        "wdt1": np.ascontiguousarray((w1[:, 3:6] - w1[:, 0:3]).T),
        "wnt2": np.ascontiguousarray(w2[:, 0:64].T),
        "wdt2": np.ascontiguousarray((w2[:, 64:128] - w2[:, 0:64]).T),
        "w3t": np.ascontiguousarray(w3.T),
        "g1c": np.asarray(g1, np.float32).reshape(64, 1),
        "b1c": np.asarray(b1, np.float32).reshape(64, 1),
        "g2c": np.ascontiguousarray(np.asarray(g2, np.float32).reshape(2, 128).T),
        "b2c": np.ascontiguousarray(np.asarray(b2, np.float32).reshape(2, 128).T),
        "g3c": np.ascontiguousarray(np.asarray(g3, np.float32).reshape(4, 128).T),
        "b3c": np.ascontiguousarray(np.asarray(b3, np.float32).reshape(4, 128).T),
        "eye": np.eye(128, dtype=np.float32),
    }
    in_maps = []
    for c in range(NC):
        m = dict(common)
        m["xs"] = np.ascontiguousarray(xs_all[c * S:(c + 1) * S])
        in_maps.append(m)

    trace = os.environ.get("KERNEL_TRACE") == "1"
    res = run_bass_kernel_spmd(nc, in_maps, core_ids=list(range(NC)), trace=trace)
    if trace:
        _cached["exec_time_ns"] = res.exec_time_ns
    outs = np.concatenate([res.results[c]["out"] for c in range(NC)], axis=0)
    return outs.reshape(B, P, 1024)


# revision 37
# speedup vs baseline: 346.7748x; 346.7748x over previous
"""DGCNN forward on 8 Trainium2 NeuronCores (Bass/Tile), data-parallel.

Math restructuring vs the reference (exact equivalences):
  * Edge-conv z_iq = U[:, idx(i,q)] + V[:, i], U = Wn X, V = (Wc-Wn) X.
  * BN+lrelu is monotone per channel (g > 0) so max_q commutes:
    x_out = lrelu(bn(max_q z)). BN stats over all N*K edges via
      S1 = deg.U + K 1.V ;  S2 = deg.U^2 + 2 sum_c U.(V^T A) + K 1.V^2
    with deg/cross from the 0/1 knn adjacency A (PE matmuls).
  * knn ranking via s_ij = x_i.x_j - |x_j|^2/2 (monotone transform of
    the reference's neg_dist per row).
  * Layer-3 max-pool commutes with bn+lrelu likewise (max_i z3 first).
  * Only BN statistics cross cores -> 3 small AllReduces.
"""
import os
import numpy as np

import concourse.bass as bass
import concourse.bacc as bacc
import concourse.mybir as mybir
import concourse.tile as tile
from concourse import library_config
from concourse.bass_utils import run_bass_kernel_spmd

F32 = mybir.dt.float32
F32R = mybir.dt.float32r
BF16 = mybir.dt.bfloat16
U16 = mybir.dt.uint16
I16 = mybir.dt.int16

NC = 8
S = 8
N = 512
K = 20
EPS = 1e-5
SLOPE = 0.2
NEG = -1e30

AOP = mybir.AluOpType
AFT = mybir.ActivationFunctionType
AXL = mybir.AxisListType

L2_BF16 = os.environ.get("KERNEL_L2F32") != "1"
PHASES = int(os.environ.get("KERNEL_PHASES", "6"))


def build_nc():
    nc = bacc.Bacc("TRN2", target_bir_lowering=False, debug=False, num_devices=NC)

    xs = nc.dram_tensor("xs", [S, N, 3], F32, kind="ExternalInput").ap()
    wnt1 = nc.dram_tensor("wnt1", [3, 64], F32, kind="ExternalInput").ap()
    wdt1 = nc.dram_tensor("wdt1", [3, 64], F32, kind="ExternalInput").ap()
    wnt2 = nc.dram_tensor("wnt2", [64, 256], F32, kind="ExternalInput").ap()
    wdt2 = nc.dram_tensor("wdt2", [64, 256], F32, kind="ExternalInput").ap()
    w3t = nc.dram_tensor("w3t", [320, 512], F32, kind="ExternalInput").ap()
    g1c = nc.dram_tensor("g1c", [64, 1], F32, kind="ExternalInput").ap()
    b1c = nc.dram_tensor("b1c", [64, 1], F32, kind="ExternalInput").ap()
    g2c = nc.dram_tensor("g2c", [128, 2], F32, kind="ExternalInput").ap()
    b2c = nc.dram_tensor("b2c", [128, 2], F32, kind="ExternalInput").ap()
    g3c = nc.dram_tensor("g3c", [128, 4], F32, kind="ExternalInput").ap()
    b3c = nc.dram_tensor("b3c", [128, 4], F32, kind="ExternalInput").ap()
    eye = nc.dram_tensor("eye", [128, 128], F32, kind="ExternalInput").ap()
    out = nc.dram_tensor("out", [S, 1024], F32, kind="ExternalOutput").ap()

    n1 = float(NC * S * N * K)
    n3 = float(NC * S * N)

    tcctx = tile.TileContext(nc)
    tc = tcctx.__enter__()
    if True:

        cp = tc.alloc_tile_pool(name="consts", bufs=1)
        dramp = tc.alloc_tile_pool(name="dram", bufs=2, space="DRAM")
        dram1 = tc.alloc_tile_pool(name="dram1", bufs=1, space="DRAM")

        eye_s = cp.tile([128, 128], F32, tag="eye")
        nc.sync.dma_start(eye_s[:], eye[:])
        eye_b = cp.tile([128, 128], BF16, tag="eyeb")
        nc.vector.tensor_copy(eye_b[:], eye_s[:])
        w1n_s = cp.tile([3, 64], F32, tag="w1n"); nc.sync.dma_start(w1n_s[:], wnt1[:])
        w1d_s = cp.tile([3, 64], F32, tag="w1d"); nc.sync.dma_start(w1d_s[:], wdt1[:])
        w2n_s = cp.tile([64, 256], F32, tag="w2n"); nc.sync.dma_start(w2n_s[:], wnt2[:])
        w2d_s = cp.tile([64, 256], F32, tag="w2d"); nc.sync.dma_start(w2d_s[:], wdt2[:])
        w3ts = []
        for kc, (p0, pn) in enumerate(((0, 64), (64, 128), (192, 128))):
            t = cp.tile([pn, 512], F32, tag=f"w3t{kc}")
            nc.sync.dma_start(t[:], w3t[p0:p0 + pn, :])
            w3ts.append(t)
        g1_s = cp.tile([64, 1], F32, tag="g1"); nc.sync.dma_start(g1_s[:], g1c[:])
        b1_s = cp.tile([64, 1], F32, tag="b1"); nc.sync.dma_start(b1_s[:], b1c[:])
        g2_s = cp.tile([128, 2], F32, tag="g2"); nc.sync.dma_start(g2_s[:], g2c[:])
        b2_s = cp.tile([128, 2], F32, tag="b2"); nc.sync.dma_start(b2_s[:], b2c[:])
        g3_s = cp.tile([128, 4], F32, tag="g3"); nc.sync.dma_start(g3_s[:], g3c[:])
        b3_s = cp.tile([128, 4], F32, tag="b3"); nc.sync.dma_start(b3_s[:], b3c[:])
        neghalf = cp.tile([64, 1], F32, tag="neghalf")
        nc.vector.memset(neghalf[:], -0.5)
        onescol = cp.tile([128, 1], F32, tag="onescol")
        nc.vector.memset(onescol[:], 1.0)
        onesrow = cp.tile([1, 512], F32, tag="onesrow")
        nc.vector.memset(onesrow[:], 1.0)


        # persistent per-sample pools
        xb2p = tc.alloc_tile_pool(name="xb2", bufs=S)
        zmt1p = tc.alloc_tile_pool(name="zmt1", bufs=S)
        zmt2p = tc.alloc_tile_pool(name="zmt2", bufs=S)
        zm3p = tc.alloc_tile_pool(name="zm3", bufs=S)
        sacc = tc.alloc_tile_pool(name="sacc", bufs=1)

        xb1p = tc.alloc_tile_pool(name="xb1", bufs=2)
        wk = tc.alloc_tile_pool(name="wk", bufs=1)
        swp = tc.alloc_tile_pool(name="swp", bufs=2)
        afp = tc.alloc_tile_pool(name="afp", bufs=4)
        idxp = tc.alloc_tile_pool(name="idxp", bufs=4)
        gp = tc.alloc_tile_pool(name="gpool", bufs=2)
        usqp = tc.alloc_tile_pool(name="usqp", bufs=1)
        x2p = tc.alloc_tile_pool(name="x2p", bufs=2)
        smallp = tc.alloc_tile_pool(name="small", bufs=4)
        rowp = tc.alloc_tile_pool(name="rowp", bufs=1)
        gvsp = tc.alloc_tile_pool(name="gvsp", bufs=1)

        ps_s = tc.alloc_tile_pool(name="ps_s", bufs=1, space="PSUM")
        ps_uv = tc.alloc_tile_pool(name="ps_uv", bufs=1, space="PSUM")
        ps_gvu = tc.alloc_tile_pool(name="ps_gvu", bufs=2, space="PSUM")
        ps_misc = tc.alloc_tile_pool(name="ps_misc", bufs=1, space="PSUM")
        ps_stat = tc.alloc_tile_pool(name="ps_stat", bufs=1, space="PSUM")

        cr1 = sacc.tile([128, 1], F32, tag="cr1"); nc.vector.memset(cr1[:], 0.0)
        cr2 = sacc.tile([128, 2], F32, tag="cr2"); nc.vector.memset(cr2[:], 0.0)
        s3sq = sacc.tile([128, 4], F32, tag="s3sq"); nc.vector.memset(s3sq[:], 0.0)
        s3s1 = sacc.tile([128, 4], F32, tag="s3s1"); nc.vector.memset(s3s1[:], 0.0)
        acc1 = sacc.tile([2, 256], F32, tag="acc1"); nc.vector.memset(acc1[:], 0.0)
        acc2 = sacc.tile([66, 512], F32, tag="acc2"); nc.vector.memset(acc2[:], 0.0)

        stA = ps_stat.tile([128, 512], F32, tag="stA")
        st3 = ps_stat.tile([128, 4], F32, tag="st3")

        L2G = BF16 if L2_BF16 else F32
        LCFG = {
            1: dict(C=3, Co=64, ncc=1, gdt=F32, zdt=F32, zp=zmt1p, cr=cr1),
            2: dict(C=64, Co=256, ncc=2, gdt=L2G, zdt=L2G, zp=zmt2p, cr=cr2),
        }

        def front(s_i, layer, xb):
            cfg = LCFG[layer]
            C, Co, ncc = cfg["C"], cfg["Co"], cfg["ncc"]
            gdt = cfg["gdt"]
            wn, wd = (w1n_s, w1d_s) if layer == 1 else (w2n_s, w2d_s)
            first = (s_i == 0)
            last = (s_i == S - 1)

            # xx row; XR = [X; -xx/2]
            x2sq = wk.tile([64, 512], F32, tag="x2sq")
            nc.scalar.activation(x2sq[0:C, :], xb[0:C, :], AFT.Square)
            xxp = ps_uv.tile([1, 512], F32, tag="uv")
            nc.tensor.matmul(xxp[:], neghalf[0:C, :], x2sq[0:C, :], start=True, stop=True)
            xr = wk.tile([65, 512], F32, tag="xr")
            nc.scalar.copy(xr[0:C, :], xb[0:C, :])
            xxs = wk.tile([1, 512], F32, tag="xxs")
            nc.scalar.copy(xxs[:], xxp[:])
            nc.sync.dma_start(xr[C:C + 1, :], xxs[:])

            # s-matrix chunks + topk scan
            af = []
            idx24 = []
            for h in range(2):
                sp = ps_s.tile([128, 1024], F32, tag="spsum")
                for q in range(2):
                    mc = h * 2 + q
                    nc.tensor.matmul(
                        sp[:, q * 512:(q + 1) * 512],
                        xb[0:C + 1, mc * 128:(mc + 1) * 128],
                        xr[0:C + 1, :], start=True, stop=True)
                sw2 = swp.tile([128, 1024], F32, tag="sw")
                nc.scalar.copy(sw2[:], sp[:])
                for q in range(2):
                    sw = sw2[:, q * 512:(q + 1) * 512]
                    i24 = idxp.tile([128, 24], U16, tag="idx24")
                    m8 = smallp.tile([128, 24], F32, tag="m8")
                    for r in range(3):
                        nc.vector.max(m8[:, 8 * r:8 * r + 8], sw)
                        nc.vector.max_index(i24[:, 8 * r:8 * r + 8],
                                            m8[:, 8 * r:8 * r + 8], sw)
                        if r < 2:
                            nc.vector.match_replace(sw, m8[:, 8 * r:8 * r + 8], sw, NEG)
                    rep8 = smallp.tile([128, 8], F32, tag="rep8")
                    nc.vector.tensor_copy(rep8[:, 0:4], m8[:, 16:20])
                    nc.vector.memset(rep8[:, 4:8], NEG)
                    nc.vector.match_replace(sw, rep8[:], sw, NEG)
                    a = afp.tile([128, 512], F32, tag="af")
                    nc.vector.tensor_scalar(a[:], sw, NEG, None, AOP.is_equal)
                    af.append(a)
                    idx24.append(i24)

            # int32 index tiles; U^T rows to DRAM; 80 indirect row-gathers
            idx32 = []
            for mc in range(4):
                i32 = idxp.tile([128, 24], mybir.dt.int32, tag="idx32")
                nc.vector.tensor_copy(i32[:, 0:20], idx24[mc][:, 0:20])
                idx32.append(i32)

            usq = usqp.tile([128, 4, 4, Co], F32, tag="usq")
            utd = dramp.tile([N, Co], F32, tag=f"utd{layer}")
            for ic in range(4):
                uvp = ps_uv.tile([128, 2 * Co], F32, tag="uv")
                nc.tensor.matmul(uvp[:, 0:Co], xb[0:C, ic * 128:(ic + 1) * 128],
                                 wn[:], start=True, stop=True)
                nc.tensor.matmul(uvp[:, Co:2 * Co], xb[0:C, ic * 128:(ic + 1) * 128],
                                 wd[:], start=True, stop=True)
                nc.scalar.copy(usq[:, ic, 0:2, :], uvp[:])
                nc.scalar.activation(usq[:, ic, 2:4, :], uvp[:], AFT.Square)
                nc.sync.dma_start(utd[ic * 128:(ic + 1) * 128, :], usq[:, ic, 0, :])

            zmt = cfg["zp"].tile([128, 4, Co], F32, tag=f"zm{layer}")
            for ic in range(4):
                g = gp.tile([128, K, Co], F32, tag="g")
                for q in range(K):
                    nc.gpsimd.indirect_dma_start(
                        out=g[:, q, :], out_offset=None,
                        in_=utd[:],
                        in_offset=bass.IndirectOffsetOnAxis(
                            ap=idx32[ic][:, q:q + 1], axis=0))
                for (na, nb) in ((10, 10), (5, 5), (2, 3), (1, 2), (1, 1)):
                    nc.vector.tensor_tensor(g[:, 0:nb, :], g[:, 0:nb, :],
                                            g[:, na:na + nb, :], AOP.max)
                nc.vector.tensor_tensor(zmt[:, ic, :], g[:, 0, :],
                                        usq[:, ic, 1, :], AOP.add)

            # cross term
            for cc in range(ncc):
                pn = min(128, Co)
                up = ps_gvu.tile([128, 512], F32, tag="gvu")
                nc.tensor.matmul(up[0:pn, :], wn[:, cc * 128:cc * 128 + pn],
                                 xb[0:C, :], start=True, stop=True)
                us = gvsp.tile([128, 512], F32, tag="us")
                nc.scalar.copy(us[0:pn, :], up[0:pn, :])
                gvp = ps_gvu.tile([128, 512], F32, tag="gvu")
                for ic in range(4):
                    nc.tensor.matmul(gvp[0:pn, :],
                                     usq[:, ic, 1, cc * 128:cc * 128 + pn],
                                     af[ic][:], start=(ic == 0), stop=(ic == 3),
                                     skip_group_check=True)
                gvs = gvsp.tile([128, 512], F32, tag="gvs")
                nc.scalar.copy(gvs[0:pn, :], gvp[0:pn, :])
                crx = smallp.tile([128, 1], F32, tag="crx")
                nc.vector.scalar_tensor_tensor(gvs[0:pn, :], us[0:pn, :], 1.0,
                                               gvs[0:pn, :], AOP.mult, AOP.mult,
                                               accum_out=crx[0:pn, :])
                cr = cfg["cr"]
                nc.vector.tensor_tensor(cr[0:pn, cc:cc + 1], cr[0:pn, cc:cc + 1],
                                        crx[0:pn, :], AOP.add)

            # ---- stats ----
            degc = smallp.tile([128, 4], F32, tag="degc")
            for jc in range(4):
                dp = ps_misc.tile([128, 512], F32, tag="misc")
                for ic in range(4):
                    nc.tensor.matmul(dp[:, 0:1], af[ic][:, jc * 128:(jc + 1) * 128],
                                     onescol[:], start=(ic == 0), stop=(ic == 3),
                                     skip_group_check=True)
                nc.scalar.copy(degc[:, jc:jc + 1], dp[:, 0:1])

            # matvec stats into persistent PSUM rows
            for ic in range(4):
                lhs = smallp.tile([128, 2], F32, tag="lhs2")
                nc.vector.tensor_copy(lhs[:, 0:1], degc[:, ic:ic + 1])
                nc.vector.tensor_copy(lhs[:, 1:2], onescol[:])
                if layer == 1:
                    nc.tensor.matmul(
                        stA[0:2, 0:256], lhs[:],
                        usq[:, ic, :, :].rearrange("p a co -> p (a co)"),
                        start=(ic == 0), stop=(ic == 3),
                        skip_group_check=True)
                else:
                    nc.tensor.matmul(
                        stA[32:34, :], lhs[:],
                        usq[:, ic, 0:2, :].rearrange("p a co -> p (a co)"),
                        start=(ic == 0), stop=(ic == 3),
                        skip_group_check=True)
                    nc.tensor.matmul(
                        stA[64:66, :], lhs[:],
                        usq[:, ic, 2:4, :].rearrange("p a co -> p (a co)"),
                        start=(ic == 0), stop=(ic == 3),
                        skip_group_check=True)
            if layer == 1:
                srow = rowp.tile([2, 512], F32, tag="srow")
                nc.scalar.copy(srow[0:2, 0:256], stA[0:2, 0:256])
                nc.vector.tensor_tensor(acc1[:], acc1[:], srow[0:2, 0:256], AOP.add)
            else:
                srow = rowp.tile([66, 512], F32, tag="srow66")
                nc.scalar.copy(srow[32:34, :], stA[32:34, :])
                nc.scalar.copy(srow[64:66, :], stA[64:66, :])
                nc.vector.tensor_tensor(acc2[32:34, :], acc2[32:34, :], srow[32:34, :], AOP.add)
                nc.vector.tensor_tensor(acc2[64:66, :], acc2[64:66, :], srow[64:66, :], AOP.add)

            return zmt

        def stats_finalize(layer, gcol, bcol):
            cfg = LCFG[layer]
            Co, ncc = cfg["Co"], cfg["ncc"]
            pn = min(128, Co)
            if layer == 1:
                sl = dict(s1a=acc1[0:1, 0:64], s1b=acc1[1:2, 64:128],
                          s2a=acc1[0:1, 128:192], s2b=acc1[1:2, 192:256])
            else:
                sl = dict(s1a=acc2[32:33, 0:256], s1b=acc2[33:34, 256:512],
                          s2a=acc2[64:65, 0:256], s2b=acc2[65:66, 256:512])
            nb = 5 * Co
            bin_ = dram1.tile([nb], F32, tag=f"arin{layer}")
            bout = dram1.tile([nb], F32, tag=f"arout{layer}")
            nc.sync.dma_start(bin_[0:Co].rearrange("(o c) -> o c", o=1), sl["s1a"])
            nc.sync.dma_start(bin_[Co:2 * Co].rearrange("(o c) -> o c", o=1), sl["s1b"])
            nc.sync.dma_start(bin_[2 * Co:3 * Co].rearrange("(o c) -> o c", o=1), sl["s2a"])
            nc.sync.dma_start(bin_[3 * Co:4 * Co].rearrange("(o c) -> o c", o=1), sl["s2b"])
            nc.sync.dma_start(
                bin_[4 * Co:5 * Co].rearrange("(cc p) -> p cc", p=pn),
                cfg["cr"][0:pn, 0:ncc])
            nc.gpsimd.collective_compute(
                "AllReduce", AOP.add, replica_groups=[list(range(NC))],
                ins=[bin_.opt()], outs=[bout.opt()])
            r1a = rowp.tile([1, 256], F32, tag="r1a")
            r1b = rowp.tile([1, 256], F32, tag="r1b")
            r2a = rowp.tile([1, 256], F32, tag="r2a")
            r2b = rowp.tile([1, 256], F32, tag="r2b")
            rcr = rowp.tile([1, 256], F32, tag="rcr")
            for t, o in ((r1a, 0), (r1b, 1), (r2a, 2), (r2b, 3), (rcr, 4)):
                nc.sync.dma_start(t[0:1, 0:Co],
                                  bout[o * Co:(o + 1) * Co].rearrange("(o c) -> o c", o=1))
            mean = rowp.tile([1, 256], F32, tag="meanr")
            nc.vector.scalar_tensor_tensor(mean[0:1, 0:Co], r1b[0:1, 0:Co], float(K),
                                           r1a[0:1, 0:Co], AOP.mult, AOP.add)
            nc.vector.tensor_scalar(mean[0:1, 0:Co], mean[0:1, 0:Co], 1.0 / n1,
                                    None, AOP.mult)
            ex2 = rowp.tile([1, 256], F32, tag="ex2r")
            nc.vector.scalar_tensor_tensor(ex2[0:1, 0:Co], rcr[0:1, 0:Co], 2.0,
                                           r2a[0:1, 0:Co], AOP.mult, AOP.add)
            nc.vector.scalar_tensor_tensor(ex2[0:1, 0:Co], r2b[0:1, 0:Co], float(K),
                                           ex2[0:1, 0:Co], AOP.mult, AOP.add)
            nc.vector.tensor_scalar(ex2[0:1, 0:Co], ex2[0:1, 0:Co], 1.0 / n1,
                                    None, AOP.mult)
            var = rowp.tile([1, 256], F32, tag="varr")
            nc.vector.scalar_tensor_tensor(var[0:1, 0:Co], mean[0:1, 0:Co], 1.0,
                                           mean[0:1, 0:Co], AOP.mult, AOP.mult)
            nc.vector.tensor_tensor(var[0:1, 0:Co], ex2[0:1, 0:Co], var[0:1, 0:Co],
                                    AOP.subtract)
            nc.vector.tensor_scalar(var[0:1, 0:Co], var[0:1, 0:Co], EPS, None, AOP.add)
            rec = rowp.tile([1, 256], F32, tag="recr")
            nc.vector.reciprocal(rec[0:1, 0:Co], var[0:1, 0:Co])
            rsq = rowp.tile([1, 256], F32, tag="rsqr")
            nc.scalar.activation(rsq[0:1, 0:Co], rec[0:1, 0:Co], AFT.Sqrt)
            sb = dram1.tile([2 * Co], F32, tag=f"sb{layer}")
            nc.sync.dma_start(sb[0:Co].rearrange("(o c) -> o c", o=1), rsq[0:1, 0:Co])
            nc.sync.dma_start(sb[Co:2 * Co].rearrange("(o c) -> o c", o=1), mean[0:1, 0:Co])
            rsqc = smallp.tile([128, 2], F32, tag=f"rsqc{layer}")
            meanc = smallp.tile([128, 2], F32, tag=f"meanc{layer}")
            nc.sync.dma_start(rsqc[0:pn, 0:ncc],
                              sb[0:Co].rearrange("(cc p) -> p cc", p=pn))
            nc.sync.dma_start(meanc[0:pn, 0:ncc],
                              sb[Co:2 * Co].rearrange("(cc p) -> p cc", p=pn))
            scc = sacc.tile([128, 2], F32, tag=f"scc{layer}")
            shc = sacc.tile([128, 2], F32, tag=f"shc{layer}")
            nc.vector.tensor_tensor(scc[0:pn, 0:ncc], gcol[0:pn, 0:ncc],
                                    rsqc[0:pn, 0:ncc], AOP.mult)
            nc.vector.tensor_tensor(shc[0:pn, 0:ncc], meanc[0:pn, 0:ncc],
                                    scc[0:pn, 0:ncc], AOP.mult)
            nc.vector.tensor_tensor(shc[0:pn, 0:ncc], bcol[0:pn, 0:ncc],
                                    shc[0:pn, 0:ncc], AOP.subtract)
            return scc, shc


        def emit_lrelu(dst, src, sc_ap, sh_ap, pn, accum=None):
            """dst = lrelu(src*sc + sh), slope 0.2; optional accum_out on final op."""
            tt = wk.tile([128, 512], F32, tag="lr_t")
            rr = wk.tile([128, 512], F32, tag="lr_r")
            nc.scalar.activation(tt[0:pn, :], src, AFT.Identity, bias=sh_ap, scale=sc_ap)
            nc.scalar.activation(rr[0:pn, :], src, AFT.Relu, bias=sh_ap, scale=sc_ap)
            nc.vector.scalar_tensor_tensor(rr[0:pn, :], rr[0:pn, :], 4.0, tt[0:pn, :],
                                           AOP.mult, AOP.add)
            if accum is None:
                nc.vector.tensor_scalar(dst, rr[0:pn, :], 0.2, None, AOP.mult)
            else:
                nc.vector.tensor_scalar(dst, rr[0:pn, :], 0.2, 0.0, AOP.mult,
                                        AOP.add, accum_out=accum)

        # ===== phase 1 =====
        zm1 = []
        for s_i in range(S):
            xb = xb1p.tile([4, 512], F32, tag="xb1")
            nc.sync.dma_start(xb[0:3, :], xs[s_i].rearrange("n d -> d n"))
            nc.sync.dma_start(xb[3:4, :], onesrow[:])
            zm1.append(front(s_i, 1, xb))

        if PHASES >= 2:
            sc1, sh1 = stats_finalize(1, g1_s, b1_s)

        # ===== phase 2 =====
        zm2 = []
        xb2s = []
        for s_i in range(S if PHASES >= 3 else 0):
            px = ps_misc.tile([128, 512], F32, tag="misc")
            for ic in range(4):
                nc.tensor.matmul(px[0:64, ic * 128:(ic + 1) * 128], zm1[s_i][:, ic, :],
                                 eye_s[:], is_transpose=True, start=True, stop=True)
            xb2 = xb2p.tile([65, 512], F32, tag="xb2")
            emit_lrelu(xb2[0:64, :], px[0:64, :], sc1[0:64, 0:1], sh1[0:64, 0:1], 64)
            nc.sync.dma_start(xb2[64:65, :], onesrow[:])
            xb2s.append(xb2)
            zm2.append(front(s_i, 2, xb2))

        if PHASES >= 4:
            sc2, sh2 = stats_finalize(2, g2_s, b2_s)

        def build_x2(s_i, cc):
            px = ps_misc.tile([128, 512], F32, tag="misc")
            for ic in range(4):
                nc.tensor.matmul(px[:, ic * 128:(ic + 1) * 128],
                                 zm2[s_i][:, ic, cc * 128:(cc + 1) * 128],
                                 eye_s[:], is_transpose=True, start=True, stop=True)
            x2 = x2p.tile([128, 512], F32, tag="x2")
            return px[:], x2

        # ===== phase 3 =====
        zm3s = []
        for s_i in range(S if PHASES >= 5 else 0):
            catsum = smallp.tile([128, 3], F32, tag="catsum")
            x2ab = []
            for cc in range(2):
                px, x2 = build_x2(s_i, cc)
                emit_lrelu(x2[:], px[:], sc2[:, cc:cc + 1], sh2[:, cc:cc + 1], 128,
                           accum=catsum[:, 1 + cc:2 + cc])
                x2ab.append(x2)
            x1scr = wk.tile([64, 512], F32, tag="x1scr")
            nc.scalar.activation(x1scr[:], xb2s[s_i][0:64, :], AFT.Copy,
                                 accum_out=catsum[0:64, 0:1])
            csl = [(catsum[0:64, 0:1], 0), (catsum[0:128, 1:2], 1),
                   (catsum[0:128, 2:3], 2)]
            for cc in range(4):
                for kci, (cs, wi) in enumerate(csl):
                    nc.tensor.matmul(st3[:, cc:cc + 1],
                                     w3ts[wi][:, cc * 128:(cc + 1) * 128], cs,
                                     start=(kci == 0), stop=(kci == 2),
                                     skip_group_check=True)
            st3row = smallp.tile([128, 4], F32, tag="st3row")
            nc.scalar.copy(st3row[:], st3[:])
            nc.vector.tensor_tensor(s3s1[:], s3s1[:], st3row[:], AOP.add)
            zm3 = zm3p.tile([128, 4], F32, tag="zm3")
            rhss = [xb2s[s_i][0:64, :], x2ab[0][:], x2ab[1][:]]
            for cc in range(4):
                zp = ps_s.tile([128, 1024], F32, tag="spsum")
                for kci, rhs in enumerate(rhss):
                    nc.tensor.matmul(zp[:, 0:512],
                                     w3ts[kci][:, cc * 128:(cc + 1) * 128],
                                     rhs,
                                     start=(kci == 0), stop=(kci == 2))
                nc.vector.tensor_reduce(zm3[:, cc:cc + 1], zp[:, 0:512], AXL.X, AOP.max)
                sqs = wk.tile([128, 512], F32, tag="sqscr")
                sqa = smallp.tile([128, 1], F32, tag="sqa")
                nc.scalar.activation(sqs[:], zp[:, 0:512], AFT.Square, accum_out=sqa[:])
                nc.vector.tensor_tensor(s3sq[:, cc:cc + 1], s3sq[:, cc:cc + 1],
                                        sqa[:], AOP.add)
            zm3s.append(zm3)

        # stats3 AR
        if PHASES < 5:
            for pool in (ps_stat, ps_misc, ps_gvu, ps_uv, ps_s, gvsp, rowp, smallp,
                         x2p, usqp, gp, idxp, afp, swp, wk, xb1p, sacc, zm3p, zmt2p,
                         zmt1p, xb2p, dram1, dramp, cp):
                pool.release()
            tcctx.__exit__(None, None, None)
            nc.compile()
            return nc
        s3rows = rowp.tile([128, 8], F32, tag="s3rows")
        nc.vector.tensor_copy(s3rows[:, 0:4], s3s1[:])
        nc.vector.tensor_copy(s3rows[:, 4:8], s3sq[:])
        b3i = dram1.tile([1024], F32, tag="ar3in")
        b3o = dram1.tile([1024], F32, tag="ar3out")
        nc.sync.dma_start(b3i[:].rearrange("(p w) -> p w", p=128), s3rows[:])
        nc.gpsimd.collective_compute("AllReduce", AOP.add,
                                     replica_groups=[list(range(NC))],
                                     ins=[b3i.opt()], outs=[b3o.opt()])
        s3r = rowp.tile([128, 8], F32, tag="s3r")
        nc.sync.dma_start(s3r[:], b3o[:].rearrange("(p w) -> p w", p=128))
        mean3 = sacc.tile([128, 4], F32, tag="mean3")
        nc.vector.tensor_scalar(mean3[:], s3r[:, 0:4], 1.0 / n3, None, AOP.mult)
        var3 = sacc.tile([128, 4], F32, tag="var3")
        nc.vector.tensor_scalar(var3[:], s3r[:, 4:8], 1.0 / n3, None, AOP.mult)
        m3sq = smallp.tile([128, 4], F32, tag="m3sq")
        nc.vector.tensor_tensor(m3sq[:], mean3[:], mean3[:], AOP.mult)
        nc.vector.tensor_tensor(var3[:], var3[:], m3sq[:], AOP.subtract)
        nc.vector.tensor_scalar(var3[:], var3[:], EPS, None, AOP.add)
        rec3 = sacc.tile([128, 4], F32, tag="rec3")
        nc.vector.reciprocal(rec3[:], var3[:])
        sc3 = sacc.tile([128, 4], F32, tag="sc3")
        nc.scalar.activation(sc3[:], rec3[:], AFT.Sqrt)
        nc.vector.tensor_tensor(sc3[:], sc3[:], g3_s[:], AOP.mult)
        sh3 = sacc.tile([128, 4], F32, tag="sh3")
        nc.vector.tensor_tensor(sh3[:], mean3[:], sc3[:], AOP.mult)
        nc.vector.tensor_tensor(sh3[:], b3_s[:], sh3[:], AOP.subtract)

        # ===== phase 4 =====
        for s_i in range(S):
            x2ab = []
            for cc in range(2):
                px, x2 = build_x2(s_i, cc)
                emit_lrelu(x2[:], px[:], sc2[:, cc:cc + 1], sh2[:, cc:cc + 1], 128)
                x2ab.append(x2)
            rhss = [xb2s[s_i][0:64, :], x2ab[0][:], x2ab[1][:]]
            hsum = smallp.tile([128, 4], F32, tag="hsum")
            for cc in range(4):
                zp = ps_s.tile([128, 1024], F32, tag="spsum")
                for kci, rhs in enumerate(rhss):
                    nc.tensor.matmul(zp[:, 0:512],
                                     w3ts[kci][:, cc * 128:(cc + 1) * 128],
                                     rhs,
                                     start=(kci == 0), stop=(kci == 2))
                hscr = wk.tile([128, 512], F32, tag="hscr")
                emit_lrelu(hscr[:], zp[:, 0:512], sc3[:, cc:cc + 1], sh3[:, cc:cc + 1],
                           128, accum=hsum[:, cc:cc + 1])
            # pooled max via commuted bn+lrelu on zm3
            t = smallp.tile([128, 4], F32, tag="tmx")
            nc.vector.tensor_tensor(t[:], zm3s[s_i][:], sc3[:], AOP.mult)
            nc.vector.tensor_tensor(t[:], t[:], sh3[:], AOP.add)
            u = smallp.tile([128, 4], F32, tag="umx")
            nc.vector.tensor_scalar(u[:], t[:], SLOPE, None, AOP.mult)
            hmax = smallp.tile([128, 4], F32, tag="hmax")
            nc.vector.tensor_tensor(hmax[:], t[:], u[:], AOP.max)
            hmean = smallp.tile([128, 4], F32, tag="hmean")
            nc.vector.tensor_scalar(hmean[:], hsum[:], 1.0 / N, None, AOP.mult)
            nc.sync.dma_start(out[s_i, 0:512].rearrange("(cc p) -> p cc", p=128),
                              hmax[:])
            nc.sync.dma_start(out[s_i, 512:1024].rearrange("(cc p) -> p cc", p=128),
                              hmean[:])

        for pool in (ps_stat, ps_misc, ps_gvu, ps_uv, ps_s, gvsp, rowp, smallp,
                     x2p, usqp, gp, idxp, afp, swp, wk, xb1p, sacc, zm3p, zmt2p,
                     zmt1p, xb2p, dram1, dramp, cp):
            pool.release()
    tcctx.__exit__(None, None, None)

    nc.compile()
    return nc


_cached = {}


def kernel(x, w1, g1, b1, w2, g2, b2, w3, g3, b3):
    x = np.asarray(x, dtype=np.float32)
    B, P, Np, D = x.shape
    xs_all = x.reshape(B * P, Np, D)
    w1 = np.asarray(w1, np.float32)
    w2 = np.asarray(w2, np.float32)
    w3 = np.asarray(w3, np.float32)

    if "nc" not in _cached:
        _cached["nc"] = build_nc()
    nc = _cached["nc"]

    common = {
        "wnt1": np.ascontiguousarray(w1[:, 0:3].T),
        "wdt1": np.ascontiguousarray((w1[:, 3:6] - w1[:, 0:3]).T),
        "wnt2": np.ascontiguousarray(w2[:, 0:64].T),
        "wdt2": np.ascontiguousarray((w2[:, 64:128] - w2[:, 0:64]).T),
        "w3t": np.ascontiguousarray(w3.T),
        "g1c": np.asarray(g1, np.float32).reshape(64, 1),
        "b1c": np.asarray(b1, np.float32).reshape(64, 1),
        "g2c": np.ascontiguousarray(np.asarray(g2, np.float32).reshape(2, 128).T),
        "b2c": np.ascontiguousarray(np.asarray(b2, np.float32).reshape(2, 128).T),
        "g3c": np.ascontiguousarray(np.asarray(g3, np.float32).reshape(4, 128).T),
        "b3c": np.ascontiguousarray(np.asarray(b3, np.float32).reshape(4, 128).T),
        "eye": np.eye(128, dtype=np.float32),
    }
    in_maps = []
    for c in range(NC):
        m = dict(common)
        m["xs"] = np.ascontiguousarray(xs_all[c * S:(c + 1) * S])
        in_maps.append(m)

    trace = os.environ.get("KERNEL_TRACE") == "1"
    res = run_bass_kernel_spmd(nc, in_maps, core_ids=list(range(NC)), trace=trace)
    if trace:
        _cached["exec_time_ns"] = res.exec_time_ns
    outs = np.concatenate([res.results[c]["out"] for c in range(NC)], axis=0)
    return outs.reshape(B, P, 1024)


# revision 38
# speedup vs baseline: 362.7580x; 1.0461x over previous
"""DGCNN forward on 8 Trainium2 NeuronCores (Bass/Tile), data-parallel.

Math restructuring vs the reference (exact equivalences):
  * Edge-conv z_iq = U[:, idx(i,q)] + V[:, i], U = Wn X, V = (Wc-Wn) X.
  * BN+lrelu is monotone per channel (g > 0) so max_q commutes:
    x_out = lrelu(bn(max_q z)). BN stats over all N*K edges via
      S1 = deg.U + K 1.V ;  S2 = deg.U^2 + 2 sum_c U.(V^T A) + K 1.V^2
    with deg/cross from the 0/1 knn adjacency A (PE matmuls).
  * knn ranking via s_ij = x_i.x_j - |x_j|^2/2 (monotone transform of
    the reference's neg_dist per row).
  * Layer-3 max-pool commutes with bn+lrelu likewise (max_i z3 first).
  * Only BN statistics cross cores -> 3 small AllReduces.
"""
import os
import numpy as np

import concourse.bass as bass
import concourse.bacc as bacc
import concourse.mybir as mybir
import concourse.tile as tile
from concourse import library_config
from concourse.bass_utils import run_bass_kernel_spmd

F32 = mybir.dt.float32
F32R = mybir.dt.float32r
BF16 = mybir.dt.bfloat16
U16 = mybir.dt.uint16
I16 = mybir.dt.int16

NC = 8
S = 8
N = 512
K = 20
EPS = 1e-5
SLOPE = 0.2
NEG = -1e30

AOP = mybir.AluOpType
AFT = mybir.ActivationFunctionType
AXL = mybir.AxisListType

L2_BF16 = os.environ.get("KERNEL_L2F32") != "1"
PHASES = int(os.environ.get("KERNEL_PHASES", "6"))


def build_nc():
    nc = bacc.Bacc("TRN2", target_bir_lowering=False, debug=False, num_devices=NC)

    xs = nc.dram_tensor("xs", [S, N, 3], F32, kind="ExternalInput").ap()
    wnt1 = nc.dram_tensor("wnt1", [3, 64], F32, kind="ExternalInput").ap()
    wdt1 = nc.dram_tensor("wdt1", [3, 64], F32, kind="ExternalInput").ap()
    wnt2 = nc.dram_tensor("wnt2", [64, 256], F32, kind="ExternalInput").ap()
    wdt2 = nc.dram_tensor("wdt2", [64, 256], F32, kind="ExternalInput").ap()
    w3t = nc.dram_tensor("w3t", [320, 512], F32, kind="ExternalInput").ap()
    g1c = nc.dram_tensor("g1c", [64, 1], F32, kind="ExternalInput").ap()
    b1c = nc.dram_tensor("b1c", [64, 1], F32, kind="ExternalInput").ap()
    g2c = nc.dram_tensor("g2c", [128, 2], F32, kind="ExternalInput").ap()
    b2c = nc.dram_tensor("b2c", [128, 2], F32, kind="ExternalInput").ap()
    g3c = nc.dram_tensor("g3c", [128, 4], F32, kind="ExternalInput").ap()
    b3c = nc.dram_tensor("b3c", [128, 4], F32, kind="ExternalInput").ap()
    eye = nc.dram_tensor("eye", [128, 128], F32, kind="ExternalInput").ap()
    out = nc.dram_tensor("out", [S, 1024], F32, kind="ExternalOutput").ap()

    n1 = float(NC * S * N * K)
    n3 = float(NC * S * N)

    tcctx = tile.TileContext(nc)
    tc = tcctx.__enter__()
    if True:

        cp = tc.alloc_tile_pool(name="consts", bufs=1)
        dramp = tc.alloc_tile_pool(name="dram", bufs=2, space="DRAM")
        dram1 = tc.alloc_tile_pool(name="dram1", bufs=1, space="DRAM")

        eye_s = cp.tile([128, 128], F32, tag="eye")
        nc.sync.dma_start(eye_s[:], eye[:])
        eye_b = cp.tile([128, 128], BF16, tag="eyeb")
        nc.vector.tensor_copy(eye_b[:], eye_s[:])
        w1n_s = cp.tile([3, 64], F32, tag="w1n"); nc.sync.dma_start(w1n_s[:], wnt1[:])
        w1d_s = cp.tile([3, 64], F32, tag="w1d"); nc.sync.dma_start(w1d_s[:], wdt1[:])
        w2n_s = cp.tile([64, 256], F32, tag="w2n"); nc.sync.dma_start(w2n_s[:], wnt2[:])
        w2d_s = cp.tile([64, 256], F32, tag="w2d"); nc.sync.dma_start(w2d_s[:], wdt2[:])
        w3ts = []
        for kc, (p0, pn) in enumerate(((0, 64), (64, 128), (192, 128))):
            t = cp.tile([pn, 512], F32, tag=f"w3t{kc}")
            nc.sync.dma_start(t[:], w3t[p0:p0 + pn, :])
            w3ts.append(t)
        g1_s = cp.tile([64, 1], F32, tag="g1"); nc.sync.dma_start(g1_s[:], g1c[:])
        b1_s = cp.tile([64, 1], F32, tag="b1"); nc.sync.dma_start(b1_s[:], b1c[:])
        g2_s = cp.tile([128, 2], F32, tag="g2"); nc.sync.dma_start(g2_s[:], g2c[:])
        b2_s = cp.tile([128, 2], F32, tag="b2"); nc.sync.dma_start(b2_s[:], b2c[:])
        g3_s = cp.tile([128, 4], F32, tag="g3"); nc.sync.dma_start(g3_s[:], g3c[:])
        b3_s = cp.tile([128, 4], F32, tag="b3"); nc.sync.dma_start(b3_s[:], b3c[:])
        neghalf = cp.tile([64, 1], F32, tag="neghalf")
        nc.vector.memset(neghalf[:], -0.5)
        onescol = cp.tile([128, 1], F32, tag="onescol")
        nc.vector.memset(onescol[:], 1.0)
        onesrow = cp.tile([1, 512], F32, tag="onesrow")
        nc.vector.memset(onesrow[:], 1.0)


        # persistent per-sample pools
        xb2p = tc.alloc_tile_pool(name="xb2", bufs=S)
        zmt1p = tc.alloc_tile_pool(name="zmt1", bufs=S)
        zmt2p = tc.alloc_tile_pool(name="zmt2", bufs=S)
        zm3p = tc.alloc_tile_pool(name="zm3", bufs=S)
        sacc = tc.alloc_tile_pool(name="sacc", bufs=1)

        xb1p = tc.alloc_tile_pool(name="xb1", bufs=3)
        wk = tc.alloc_tile_pool(name="wk", bufs=1)
        swp = tc.alloc_tile_pool(name="swp", bufs=1)
        afp = tc.alloc_tile_pool(name="afp", bufs=4)
        idxp = tc.alloc_tile_pool(name="idxp", bufs=8)
        gp = tc.alloc_tile_pool(name="gpool", bufs=1)
        usqp = tc.alloc_tile_pool(name="usqp", bufs=2)
        x2p = tc.alloc_tile_pool(name="x2p", bufs=2)
        smallp = tc.alloc_tile_pool(name="small", bufs=4)
        rowp = tc.alloc_tile_pool(name="rowp", bufs=1)
        gvsp = tc.alloc_tile_pool(name="gvsp", bufs=1)

        ps_s = tc.alloc_tile_pool(name="ps_s", bufs=1, space="PSUM")
        ps_uv = tc.alloc_tile_pool(name="ps_uv", bufs=1, space="PSUM")
        ps_gvu = tc.alloc_tile_pool(name="ps_gvu", bufs=2, space="PSUM")
        ps_misc = tc.alloc_tile_pool(name="ps_misc", bufs=1, space="PSUM")
        ps_stat = tc.alloc_tile_pool(name="ps_stat", bufs=1, space="PSUM")

        cr1 = sacc.tile([128, 1], F32, tag="cr1"); nc.vector.memset(cr1[:], 0.0)
        cr2 = sacc.tile([128, 2], F32, tag="cr2"); nc.vector.memset(cr2[:], 0.0)
        s3sq = sacc.tile([128, 4], F32, tag="s3sq"); nc.vector.memset(s3sq[:], 0.0)
        s3s1 = sacc.tile([128, 4], F32, tag="s3s1"); nc.vector.memset(s3s1[:], 0.0)
        acc1 = sacc.tile([2, 256], F32, tag="acc1"); nc.vector.memset(acc1[:], 0.0)
        acc2 = sacc.tile([66, 512], F32, tag="acc2"); nc.vector.memset(acc2[:], 0.0)

        stA = ps_stat.tile([128, 512], F32, tag="stA")
        st3 = ps_stat.tile([128, 4], F32, tag="st3")

        L2G = BF16 if L2_BF16 else F32
        LCFG = {
            1: dict(C=3, Co=64, ncc=1, gdt=F32, zdt=F32, zp=zmt1p, cr=cr1),
            2: dict(C=64, Co=256, ncc=2, gdt=L2G, zdt=L2G, zp=zmt2p, cr=cr2),
        }

        def front(s_i, layer, xb):
            cfg = LCFG[layer]
            C, Co, ncc = cfg["C"], cfg["Co"], cfg["ncc"]
            gdt = cfg["gdt"]
            wn, wd = (w1n_s, w1d_s) if layer == 1 else (w2n_s, w2d_s)
            first = (s_i == 0)
            last = (s_i == S - 1)

            # xx row; XR = [X; -xx/2]
            x2sq = wk.tile([64, 512], F32, tag="x2sq")
            nc.scalar.activation(x2sq[0:C, :], xb[0:C, :], AFT.Square)
            xxp = ps_uv.tile([1, 512], F32, tag="uv")
            nc.tensor.matmul(xxp[:], neghalf[0:C, :], x2sq[0:C, :], start=True, stop=True)
            xr = wk.tile([65, 512], F32, tag="xr")
            nc.scalar.copy(xr[0:C, :], xb[0:C, :])
            xxs = wk.tile([1, 512], F32, tag="xxs")
            nc.scalar.copy(xxs[:], xxp[:])
            nc.sync.dma_start(xr[C:C + 1, :], xxs[:])

            # s-matrix chunks + topk scan
            af = []
            idx24 = []
            for h in range(2):
                sp = ps_s.tile([128, 1024], F32, tag="spsum")
                for q in range(2):
                    mc = h * 2 + q
                    nc.tensor.matmul(
                        sp[:, q * 512:(q + 1) * 512],
                        xb[0:C + 1, mc * 128:(mc + 1) * 128],
                        xr[0:C + 1, :], start=True, stop=True)
                sw2 = swp.tile([128, 1024], F32, tag="sw")
                nc.scalar.copy(sw2[:], sp[:])
                for q in range(2):
                    sw = sw2[:, q * 512:(q + 1) * 512]
                    i24 = idxp.tile([128, 24], U16, tag="idx24")
                    m8 = smallp.tile([128, 24], F32, tag="m8")
                    for r in range(3):
                        nc.vector.max(m8[:, 8 * r:8 * r + 8], sw)
                        nc.vector.max_index(i24[:, 8 * r:8 * r + 8],
                                            m8[:, 8 * r:8 * r + 8], sw)
                        if r < 2:
                            nc.vector.match_replace(sw, m8[:, 8 * r:8 * r + 8], sw, NEG)
                    rep8 = smallp.tile([128, 8], F32, tag="rep8")
                    nc.vector.tensor_copy(rep8[:, 0:4], m8[:, 16:20])
                    nc.vector.memset(rep8[:, 4:8], NEG)
                    nc.vector.match_replace(sw, rep8[:], sw, NEG)
                    a = afp.tile([128, 512], F32, tag="af")
                    nc.vector.tensor_scalar(a[:], sw, NEG, None, AOP.is_equal)
                    af.append(a)
                    idx24.append(i24)

            # int32 index tiles; U^T rows to DRAM; 80 indirect row-gathers
            idx32 = []
            for mc in range(4):
                i32 = idxp.tile([128, 24], mybir.dt.int32, tag="idx32")
                nc.vector.tensor_copy(i32[:, 0:20], idx24[mc][:, 0:20])
                idx32.append(i32)

            usq = usqp.tile([128, 4, 4, Co], F32, tag="usq")
            utd = dramp.tile([N, Co], F32, tag=f"utd{layer}")
            for ic in range(4):
                uvp = ps_uv.tile([128, 2 * Co], F32, tag="uv")
                nc.tensor.matmul(uvp[:, 0:Co], xb[0:C, ic * 128:(ic + 1) * 128],
                                 wn[:], start=True, stop=True)
                nc.tensor.matmul(uvp[:, Co:2 * Co], xb[0:C, ic * 128:(ic + 1) * 128],
                                 wd[:], start=True, stop=True)
                nc.scalar.copy(usq[:, ic, 0:2, :], uvp[:])
                nc.scalar.activation(usq[:, ic, 2:4, :], uvp[:], AFT.Square)
                nc.sync.dma_start(utd[ic * 128:(ic + 1) * 128, :], usq[:, ic, 0, :])

            zmt = cfg["zp"].tile([128, 4, Co], F32, tag=f"zm{layer}")
            for ic in range(4):
                g = gp.tile([128, K, Co], F32, tag="g")
                for q in range(K):
                    nc.gpsimd.indirect_dma_start(
                        out=g[:, q, :], out_offset=None,
                        in_=utd[:],
                        in_offset=bass.IndirectOffsetOnAxis(
                            ap=idx32[ic][:, q:q + 1], axis=0))
                for (na, nb) in ((10, 10), (5, 5), (2, 3), (1, 2), (1, 1)):
                    nc.vector.tensor_tensor(g[:, 0:nb, :], g[:, 0:nb, :],
                                            g[:, na:na + nb, :], AOP.max)
                nc.vector.tensor_tensor(zmt[:, ic, :], g[:, 0, :],
                                        usq[:, ic, 1, :], AOP.add)

            # cross term
            for cc in range(ncc):
                pn = min(128, Co)
                up = ps_gvu.tile([128, 512], F32, tag="gvu")
                nc.tensor.matmul(up[0:pn, :], wn[:, cc * 128:cc * 128 + pn],
                                 xb[0:C, :], start=True, stop=True)
                us = gvsp.tile([128, 512], F32, tag="us")
                nc.scalar.copy(us[0:pn, :], up[0:pn, :])
                gvp = ps_gvu.tile([128, 512], F32, tag="gvu")
                for ic in range(4):
                    nc.tensor.matmul(gvp[0:pn, :],
                                     usq[:, ic, 1, cc * 128:cc * 128 + pn],
                                     af[ic][:], start=(ic == 0), stop=(ic == 3),
                                     skip_group_check=True)
                gvs = gvsp.tile([128, 512], F32, tag="gvs")
                nc.scalar.copy(gvs[0:pn, :], gvp[0:pn, :])
                crx = smallp.tile([128, 1], F32, tag="crx")
                nc.vector.scalar_tensor_tensor(gvs[0:pn, :], us[0:pn, :], 1.0,
                                               gvs[0:pn, :], AOP.mult, AOP.mult,
                                               accum_out=crx[0:pn, :])
                cr = cfg["cr"]
                nc.vector.tensor_tensor(cr[0:pn, cc:cc + 1], cr[0:pn, cc:cc + 1],
                                        crx[0:pn, :], AOP.add)

            # ---- stats ----
            degc = smallp.tile([128, 4], F32, tag="degc")
            for jc in range(4):
                dp = ps_misc.tile([128, 512], F32, tag="misc")
                for ic in range(4):
                    nc.tensor.matmul(dp[:, 0:1], af[ic][:, jc * 128:(jc + 1) * 128],
                                     onescol[:], start=(ic == 0), stop=(ic == 3),
                                     skip_group_check=True)
                nc.scalar.copy(degc[:, jc:jc + 1], dp[:, 0:1])

            # matvec stats into persistent PSUM rows
            for ic in range(4):
                lhs = smallp.tile([128, 2], F32, tag="lhs2")
                nc.vector.tensor_copy(lhs[:, 0:1], degc[:, ic:ic + 1])
                nc.vector.tensor_copy(lhs[:, 1:2], onescol[:])
                if layer == 1:
                    nc.tensor.matmul(
                        stA[0:2, 0:256], lhs[:],
                        usq[:, ic, :, :].rearrange("p a co -> p (a co)"),
                        start=(ic == 0), stop=(ic == 3),
                        skip_group_check=True)
                else:
                    nc.tensor.matmul(
                        stA[32:34, :], lhs[:],
                        usq[:, ic, 0:2, :].rearrange("p a co -> p (a co)"),
                        start=(ic == 0), stop=(ic == 3),
                        skip_group_check=True)
                    nc.tensor.matmul(
                        stA[64:66, :], lhs[:],
                        usq[:, ic, 2:4, :].rearrange("p a co -> p (a co)"),
                        start=(ic == 0), stop=(ic == 3),
                        skip_group_check=True)
            if layer == 1:
                srow = rowp.tile([2, 512], F32, tag="srow")
                nc.scalar.copy(srow[0:2, 0:256], stA[0:2, 0:256])
                nc.vector.tensor_tensor(acc1[:], acc1[:], srow[0:2, 0:256], AOP.add)
            else:
                srow = rowp.tile([66, 512], F32, tag="srow66")
                nc.scalar.copy(srow[32:34, :], stA[32:34, :])
                nc.scalar.copy(srow[64:66, :], stA[64:66, :])
                nc.vector.tensor_tensor(acc2[32:34, :], acc2[32:34, :], srow[32:34, :], AOP.add)
                nc.vector.tensor_tensor(acc2[64:66, :], acc2[64:66, :], srow[64:66, :], AOP.add)

            return zmt

        def stats_finalize(layer, gcol, bcol):
            cfg = LCFG[layer]
            Co, ncc = cfg["Co"], cfg["ncc"]
            pn = min(128, Co)
            if layer == 1:
                sl = dict(s1a=acc1[0:1, 0:64], s1b=acc1[1:2, 64:128],
                          s2a=acc1[0:1, 128:192], s2b=acc1[1:2, 192:256])
            else:
                sl = dict(s1a=acc2[32:33, 0:256], s1b=acc2[33:34, 256:512],
                          s2a=acc2[64:65, 0:256], s2b=acc2[65:66, 256:512])
            nb = 5 * Co
            bin_ = dram1.tile([nb], F32, tag=f"arin{layer}")
            bout = dram1.tile([nb], F32, tag=f"arout{layer}")
            nc.sync.dma_start(bin_[0:Co].rearrange("(o c) -> o c", o=1), sl["s1a"])
            nc.sync.dma_start(bin_[Co:2 * Co].rearrange("(o c) -> o c", o=1), sl["s1b"])
            nc.sync.dma_start(bin_[2 * Co:3 * Co].rearrange("(o c) -> o c", o=1), sl["s2a"])
            nc.sync.dma_start(bin_[3 * Co:4 * Co].rearrange("(o c) -> o c", o=1), sl["s2b"])
            nc.sync.dma_start(
                bin_[4 * Co:5 * Co].rearrange("(cc p) -> p cc", p=pn),
                cfg["cr"][0:pn, 0:ncc])
            nc.gpsimd.collective_compute(
                "AllReduce", AOP.add, replica_groups=[list(range(NC))],
                ins=[bin_.opt()], outs=[bout.opt()])
            r1a = rowp.tile([1, 256], F32, tag="r1a")
            r1b = rowp.tile([1, 256], F32, tag="r1b")
            r2a = rowp.tile([1, 256], F32, tag="r2a")
            r2b = rowp.tile([1, 256], F32, tag="r2b")
            rcr = rowp.tile([1, 256], F32, tag="rcr")
            for t, o in ((r1a, 0), (r1b, 1), (r2a, 2), (r2b, 3), (rcr, 4)):
                nc.sync.dma_start(t[0:1, 0:Co],
                                  bout[o * Co:(o + 1) * Co].rearrange("(o c) -> o c", o=1))
            mean = rowp.tile([1, 256], F32, tag="meanr")
            nc.vector.scalar_tensor_tensor(mean[0:1, 0:Co], r1b[0:1, 0:Co], float(K),
                                           r1a[0:1, 0:Co], AOP.mult, AOP.add)
            nc.vector.tensor_scalar(mean[0:1, 0:Co], mean[0:1, 0:Co], 1.0 / n1,
                                    None, AOP.mult)
            ex2 = rowp.tile([1, 256], F32, tag="ex2r")
            nc.vector.scalar_tensor_tensor(ex2[0:1, 0:Co], rcr[0:1, 0:Co], 2.0,
                                           r2a[0:1, 0:Co], AOP.mult, AOP.add)
            nc.vector.scalar_tensor_tensor(ex2[0:1, 0:Co], r2b[0:1, 0:Co], float(K),
                                           ex2[0:1, 0:Co], AOP.mult, AOP.add)
            nc.vector.tensor_scalar(ex2[0:1, 0:Co], ex2[0:1, 0:Co], 1.0 / n1,
                                    None, AOP.mult)
            var = rowp.tile([1, 256], F32, tag="varr")
            nc.vector.scalar_tensor_tensor(var[0:1, 0:Co], mean[0:1, 0:Co], 1.0,
                                           mean[0:1, 0:Co], AOP.mult, AOP.mult)
            nc.vector.tensor_tensor(var[0:1, 0:Co], ex2[0:1, 0:Co], var[0:1, 0:Co],
                                    AOP.subtract)
            nc.vector.tensor_scalar(var[0:1, 0:Co], var[0:1, 0:Co], EPS, None, AOP.add)
            rec = rowp.tile([1, 256], F32, tag="recr")
            nc.vector.reciprocal(rec[0:1, 0:Co], var[0:1, 0:Co])
            rsq = rowp.tile([1, 256], F32, tag="rsqr")
            nc.scalar.activation(rsq[0:1, 0:Co], rec[0:1, 0:Co], AFT.Sqrt)
            sb = dram1.tile([2 * Co], F32, tag=f"sb{layer}")
            nc.sync.dma_start(sb[0:Co].rearrange("(o c) -> o c", o=1), rsq[0:1, 0:Co])
            nc.sync.dma_start(sb[Co:2 * Co].rearrange("(o c) -> o c", o=1), mean[0:1, 0:Co])
            rsqc = smallp.tile([128, 2], F32, tag=f"rsqc{layer}")
            meanc = smallp.tile([128, 2], F32, tag=f"meanc{layer}")
            nc.sync.dma_start(rsqc[0:pn, 0:ncc],
                              sb[0:Co].rearrange("(cc p) -> p cc", p=pn))
            nc.sync.dma_start(meanc[0:pn, 0:ncc],
                              sb[Co:2 * Co].rearrange("(cc p) -> p cc", p=pn))
            scc = sacc.tile([128, 2], F32, tag=f"scc{layer}")
            shc = sacc.tile([128, 2], F32, tag=f"shc{layer}")
            nc.vector.tensor_tensor(scc[0:pn, 0:ncc], gcol[0:pn, 0:ncc],
                                    rsqc[0:pn, 0:ncc], AOP.mult)
            nc.vector.tensor_tensor(shc[0:pn, 0:ncc], meanc[0:pn, 0:ncc],
                                    scc[0:pn, 0:ncc], AOP.mult)
            nc.vector.tensor_tensor(shc[0:pn, 0:ncc], bcol[0:pn, 0:ncc],
                                    shc[0:pn, 0:ncc], AOP.subtract)
            return scc, shc


        def emit_lrelu(dst, src, sc_ap, sh_ap, pn, accum=None):
            """dst = lrelu(src*sc + sh), slope 0.2; optional accum_out on final op."""
            tt = wk.tile([128, 512], F32, tag="lr_t")
            rr = wk.tile([128, 512], F32, tag="lr_r")
            nc.scalar.activation(tt[0:pn, :], src, AFT.Identity, bias=sh_ap, scale=sc_ap)
            nc.scalar.activation(rr[0:pn, :], src, AFT.Relu, bias=sh_ap, scale=sc_ap)
            nc.vector.scalar_tensor_tensor(rr[0:pn, :], rr[0:pn, :], 4.0, tt[0:pn, :],
                                           AOP.mult, AOP.add)
            if accum is None:
                nc.vector.tensor_scalar(dst, rr[0:pn, :], 0.2, None, AOP.mult)
            else:
                nc.vector.tensor_scalar(dst, rr[0:pn, :], 0.2, 0.0, AOP.mult,
                                        AOP.add, accum_out=accum)

        # ===== phase 1 =====
        zm1 = []
        for s_i in range(S):
            xb = xb1p.tile([4, 512], F32, tag="xb1")
            nc.sync.dma_start(xb[0:3, :], xs[s_i].rearrange("n d -> d n"))
            nc.sync.dma_start(xb[3:4, :], onesrow[:])
            zm1.append(front(s_i, 1, xb))

        if PHASES >= 2:
            sc1, sh1 = stats_finalize(1, g1_s, b1_s)

        # ===== phase 2 =====
        zm2 = []
        xb2s = []
        for s_i in range(S if PHASES >= 3 else 0):
            px = ps_misc.tile([128, 512], F32, tag="misc")
            for ic in range(4):
                nc.tensor.matmul(px[0:64, ic * 128:(ic + 1) * 128], zm1[s_i][:, ic, :],
                                 eye_s[:], is_transpose=True, start=True, stop=True)
            xb2 = xb2p.tile([65, 512], F32, tag="xb2")
            emit_lrelu(xb2[0:64, :], px[0:64, :], sc1[0:64, 0:1], sh1[0:64, 0:1], 64)
            nc.sync.dma_start(xb2[64:65, :], onesrow[:])
            xb2s.append(xb2)
            zm2.append(front(s_i, 2, xb2))

        if PHASES >= 4:
            sc2, sh2 = stats_finalize(2, g2_s, b2_s)

        def build_x2(s_i, cc):
            px = ps_misc.tile([128, 512], F32, tag="misc")
            for ic in range(4):
                nc.tensor.matmul(px[:, ic * 128:(ic + 1) * 128],
                                 zm2[s_i][:, ic, cc * 128:(cc + 1) * 128],
                                 eye_s[:], is_transpose=True, start=True, stop=True)
            x2 = x2p.tile([128, 512], F32, tag="x2")
            return px[:], x2

        # ===== phase 3 =====
        zm3s = []
        for s_i in range(S if PHASES >= 5 else 0):
            catsum = smallp.tile([128, 3], F32, tag="catsum")
            x2ab = []
            for cc in range(2):
                px, x2 = build_x2(s_i, cc)
                emit_lrelu(x2[:], px[:], sc2[:, cc:cc + 1], sh2[:, cc:cc + 1], 128,
                           accum=catsum[:, 1 + cc:2 + cc])
                x2ab.append(x2)
            x1scr = wk.tile([64, 512], F32, tag="x1scr")
            nc.scalar.activation(x1scr[:], xb2s[s_i][0:64, :], AFT.Copy,
                                 accum_out=catsum[0:64, 0:1])
            csl = [(catsum[0:64, 0:1], 0), (catsum[0:128, 1:2], 1),
                   (catsum[0:128, 2:3], 2)]
            for cc in range(4):
                for kci, (cs, wi) in enumerate(csl):
                    nc.tensor.matmul(st3[:, cc:cc + 1],
                                     w3ts[wi][:, cc * 128:(cc + 1) * 128], cs,
                                     start=(kci == 0), stop=(kci == 2),
                                     skip_group_check=True)
            st3row = smallp.tile([128, 4], F32, tag="st3row")
            nc.scalar.copy(st3row[:], st3[:])
            nc.vector.tensor_tensor(s3s1[:], s3s1[:], st3row[:], AOP.add)
            zm3 = zm3p.tile([128, 4], F32, tag="zm3")
            rhss = [xb2s[s_i][0:64, :], x2ab[0][:], x2ab[1][:]]
            for cc in range(4):
                zp = ps_s.tile([128, 1024], F32, tag="spsum")
                for kci, rhs in enumerate(rhss):
                    nc.tensor.matmul(zp[:, 0:512],
                                     w3ts[kci][:, cc * 128:(cc + 1) * 128],
                                     rhs,
                                     start=(kci == 0), stop=(kci == 2))
                nc.vector.tensor_reduce(zm3[:, cc:cc + 1], zp[:, 0:512], AXL.X, AOP.max)
                sqs = wk.tile([128, 512], F32, tag="sqscr")
                sqa = smallp.tile([128, 1], F32, tag="sqa")
                nc.scalar.activation(sqs[:], zp[:, 0:512], AFT.Square, accum_out=sqa[:])
                nc.vector.tensor_tensor(s3sq[:, cc:cc + 1], s3sq[:, cc:cc + 1],
                                        sqa[:], AOP.add)
            zm3s.append(zm3)

        # stats3 AR
        if PHASES < 5:
            for pool in (ps_stat, ps_misc, ps_gvu, ps_uv, ps_s, gvsp, rowp, smallp,
                         x2p, usqp, gp, idxp, afp, swp, wk, xb1p, sacc, zm3p, zmt2p,
                         zmt1p, xb2p, dram1, dramp, cp):
                pool.release()
            tcctx.__exit__(None, None, None)
            nc.compile()
            return nc
        s3rows = rowp.tile([128, 8], F32, tag="s3rows")
        nc.vector.tensor_copy(s3rows[:, 0:4], s3s1[:])
        nc.vector.tensor_copy(s3rows[:, 4:8], s3sq[:])
        b3i = dram1.tile([1024], F32, tag="ar3in")
        b3o = dram1.tile([1024], F32, tag="ar3out")
        nc.sync.dma_start(b3i[:].rearrange("(p w) -> p w", p=128), s3rows[:])
        nc.gpsimd.collective_compute("AllReduce", AOP.add,
                                     replica_groups=[list(range(NC))],
                                     ins=[b3i.opt()], outs=[b3o.opt()])
        s3r = rowp.tile([128, 8], F32, tag="s3r")
        nc.sync.dma_start(s3r[:], b3o[:].rearrange("(p w) -> p w", p=128))
        mean3 = sacc.tile([128, 4], F32, tag="mean3")
        nc.vector.tensor_scalar(mean3[:], s3r[:, 0:4], 1.0 / n3, None, AOP.mult)
        var3 = sacc.tile([128, 4], F32, tag="var3")
        nc.vector.tensor_scalar(var3[:], s3r[:, 4:8], 1.0 / n3, None, AOP.mult)
        m3sq = smallp.tile([128, 4], F32, tag="m3sq")
        nc.vector.tensor_tensor(m3sq[:], mean3[:], mean3[:], AOP.mult)
        nc.vector.tensor_tensor(var3[:], var3[:], m3sq[:], AOP.subtract)
        nc.vector.tensor_scalar(var3[:], var3[:], EPS, None, AOP.add)
        rec3 = sacc.tile([128, 4], F32, tag="rec3")
        nc.vector.reciprocal(rec3[:], var3[:])
        sc3 = sacc.tile([128, 4], F32, tag="sc3")
        nc.scalar.activation(sc3[:], rec3[:], AFT.Sqrt)
        nc.vector.tensor_tensor(sc3[:], sc3[:], g3_s[:], AOP.mult)
        sh3 = sacc.tile([128, 4], F32, tag="sh3")
        nc.vector.tensor_tensor(sh3[:], mean3[:], sc3[:], AOP.mult)
        nc.vector.tensor_tensor(sh3[:], b3_s[:], sh3[:], AOP.subtract)

        # ===== phase 4 =====
        for s_i in range(S):
            x2ab = []
            for cc in range(2):
                px, x2 = build_x2(s_i, cc)
                emit_lrelu(x2[:], px[:], sc2[:, cc:cc + 1], sh2[:, cc:cc + 1], 128)
                x2ab.append(x2)
            rhss = [xb2s[s_i][0:64, :], x2ab[0][:], x2ab[1][:]]
            hsum = smallp.tile([128, 4], F32, tag="hsum")
            for cc in range(4):
                zp = ps_s.tile([128, 1024], F32, tag="spsum")
                for kci, rhs in enumerate(rhss):
                    nc.tensor.matmul(zp[:, 0:512],
                                     w3ts[kci][:, cc * 128:(cc + 1) * 128],
                                     rhs,
                                     start=(kci == 0), stop=(kci == 2))
                hscr = wk.tile([128, 512], F32, tag="hscr")
                emit_lrelu(hscr[:], zp[:, 0:512], sc3[:, cc:cc + 1], sh3[:, cc:cc + 1],
                           128, accum=hsum[:, cc:cc + 1])
            # pooled max via commuted bn+lrelu on zm3
            t = smallp.tile([128, 4], F32, tag="tmx")
            nc.vector.tensor_tensor(t[:], zm3s[s_i][:], sc3[:], AOP.mult)
            nc.vector.tensor_tensor(t[:], t[:], sh3[:], AOP.add)
            u = smallp.tile([128, 4], F32, tag="umx")
            nc.vector.tensor_scalar(u[:], t[:], SLOPE, None, AOP.mult)
            hmax = smallp.tile([128, 4], F32, tag="hmax")
            nc.vector.tensor_tensor(hmax[:], t[:], u[:], AOP.max)
            hmean = smallp.tile([128, 4], F32, tag="hmean")
            nc.vector.tensor_scalar(hmean[:], hsum[:], 1.0 / N, None, AOP.mult)
            nc.sync.dma_start(out[s_i, 0:512].rearrange("(cc p) -> p cc", p=128),
                              hmax[:])
            nc.sync.dma_start(out[s_i, 512:1024].rearrange("(cc p) -> p cc", p=128),
                              hmean[:])

        for pool in (ps_stat, ps_misc, ps_gvu, ps_uv, ps_s, gvsp, rowp, smallp,
                     x2p, usqp, gp, idxp, afp, swp, wk, xb1p, sacc, zm3p, zmt2p,
                     zmt1p, xb2p, dram1, dramp, cp):
            pool.release()
    tcctx.__exit__(None, None, None)

    nc.compile()
    return nc


_cached = {}


def kernel(x, w1, g1, b1, w2, g2, b2, w3, g3, b3):
    x = np.asarray(x, dtype=np.float32)
    B, P, Np, D = x.shape
    xs_all = x.reshape(B * P, Np, D)
    w1 = np.asarray(w1, np.float32)
    w2 = np.asarray(w2, np.float32)
    w3 = np.asarray(w3, np.float32)

    if "nc" not in _cached:
        _cached["nc"] = build_nc()
    nc = _cached["nc"]

    common = {
        "wnt1": np.ascontiguousarray(w1[:, 0:3].T),
        "wdt1": np.ascontiguousarray((w1[:, 3:6] - w1[:, 0:3]).T),
        "wnt2": np.ascontiguousarray(w2[:, 0:64].T),
        "wdt2": np.ascontiguousarray((w2[:, 64:128] - w2[:, 0:64]).T),
        "w3t": np.ascontiguousarray(w3.T),
        "g1c": np.asarray(g1, np.float32).reshape(64, 1),
        "b1c": np.asarray(b1, np.float32).reshape(64, 1),
        "g2c": np.ascontiguousarray(np.asarray(g2, np.float32).reshape(2, 128).T),
        "b2c": np.ascontiguousarray(np.asarray(b2, np.float32).reshape(2, 128).T),
        "g3c": np.ascontiguousarray(np.asarray(g3, np.float32).reshape(4, 128).T),
        "b3c": np.ascontiguousarray(np.asarray(b3, np.float32).reshape(4, 128).T),
        "eye": np.eye(128, dtype=np.float32),
    }
    in_maps = []
    for c in range(NC):
        m = dict(common)
        m["xs"] = np.ascontiguousarray(xs_all[c * S:(c + 1) * S])
        in_maps.append(m)

    trace = os.environ.get("KERNEL_TRACE") == "1"
    res = run_bass_kernel_spmd(nc, in_maps, core_ids=list(range(NC)), trace=trace)
    if trace:
        _cached["exec_time_ns"] = res.exec_time_ns
    outs = np.concatenate([res.results[c]["out"] for c in range(NC)], axis=0)
    return outs.reshape(B, P, 1024)


# revision 39
# speedup vs baseline: 377.5962x; 1.0409x over previous
"""DGCNN forward on 8 Trainium2 NeuronCores (Bass/Tile), data-parallel.

Math restructuring vs the reference (exact equivalences):
  * Edge-conv z_iq = U[:, idx(i,q)] + V[:, i], U = Wn X, V = (Wc-Wn) X.
  * BN+lrelu is monotone per channel (g > 0) so max_q commutes:
    x_out = lrelu(bn(max_q z)). BN stats over all N*K edges via
      S1 = deg.U + K 1.V ;  S2 = deg.U^2 + 2 sum_c U.(V^T A) + K 1.V^2
    with deg/cross from the 0/1 knn adjacency A (PE matmuls).
  * knn ranking via s_ij = x_i.x_j - |x_j|^2/2 (monotone transform of
    the reference's neg_dist per row).
  * Layer-3 max-pool commutes with bn+lrelu likewise (max_i z3 first).
  * Only BN statistics cross cores -> 3 small AllReduces.
"""
import os
import numpy as np

import concourse.bass as bass
import concourse.bacc as bacc
import concourse.mybir as mybir
import concourse.tile as tile
from concourse import library_config
from concourse.bass_utils import run_bass_kernel_spmd

F32 = mybir.dt.float32
F32R = mybir.dt.float32r
BF16 = mybir.dt.bfloat16
U16 = mybir.dt.uint16
I16 = mybir.dt.int16

NC = 8
S = 8
N = 512
K = 20
EPS = 1e-5
SLOPE = 0.2
NEG = -1e30

AOP = mybir.AluOpType
AFT = mybir.ActivationFunctionType
AXL = mybir.AxisListType

L2_BF16 = os.environ.get("KERNEL_L2F32") != "1"
PHASES = int(os.environ.get("KERNEL_PHASES", "6"))


def build_nc():
    nc = bacc.Bacc("TRN2", target_bir_lowering=False, debug=False, num_devices=NC)

    xs = nc.dram_tensor("xs", [S, N, 3], F32, kind="ExternalInput").ap()
    wnt1 = nc.dram_tensor("wnt1", [3, 64], F32, kind="ExternalInput").ap()
    wdt1 = nc.dram_tensor("wdt1", [3, 64], F32, kind="ExternalInput").ap()
    wnt2 = nc.dram_tensor("wnt2", [64, 256], F32, kind="ExternalInput").ap()
    wdt2 = nc.dram_tensor("wdt2", [64, 256], F32, kind="ExternalInput").ap()
    w3t = nc.dram_tensor("w3t", [320, 512], F32, kind="ExternalInput").ap()
    g1c = nc.dram_tensor("g1c", [64, 1], F32, kind="ExternalInput").ap()
    b1c = nc.dram_tensor("b1c", [64, 1], F32, kind="ExternalInput").ap()
    g2c = nc.dram_tensor("g2c", [128, 2], F32, kind="ExternalInput").ap()
    b2c = nc.dram_tensor("b2c", [128, 2], F32, kind="ExternalInput").ap()
    g3c = nc.dram_tensor("g3c", [128, 4], F32, kind="ExternalInput").ap()
    b3c = nc.dram_tensor("b3c", [128, 4], F32, kind="ExternalInput").ap()
    eye = nc.dram_tensor("eye", [128, 128], F32, kind="ExternalInput").ap()
    out = nc.dram_tensor("out", [S, 1024], F32, kind="ExternalOutput").ap()

    n1 = float(NC * S * N * K)
    n3 = float(NC * S * N)

    tcctx = tile.TileContext(nc)
    tc = tcctx.__enter__()
    if True:

        cp = tc.alloc_tile_pool(name="consts", bufs=1)
        dramp = tc.alloc_tile_pool(name="dram", bufs=2, space="DRAM")
        dram1 = tc.alloc_tile_pool(name="dram1", bufs=1, space="DRAM")

        eye_s = cp.tile([128, 128], F32, tag="eye")
        nc.sync.dma_start(eye_s[:], eye[:])
        eye_b = cp.tile([128, 128], BF16, tag="eyeb")
        nc.vector.tensor_copy(eye_b[:], eye_s[:])
        w1n_s = cp.tile([3, 64], F32, tag="w1n"); nc.sync.dma_start(w1n_s[:], wnt1[:])
        w1d_s = cp.tile([3, 64], F32, tag="w1d"); nc.sync.dma_start(w1d_s[:], wdt1[:])
        w2n_s = cp.tile([64, 256], F32, tag="w2n"); nc.sync.dma_start(w2n_s[:], wnt2[:])
        w2d_s = cp.tile([64, 256], F32, tag="w2d"); nc.sync.dma_start(w2d_s[:], wdt2[:])
        w3ts = []
        for kc, (p0, pn) in enumerate(((0, 64), (64, 128), (192, 128))):
            t = cp.tile([pn, 512], F32, tag=f"w3t{kc}")
            nc.sync.dma_start(t[:], w3t[p0:p0 + pn, :])
            w3ts.append(t)
        g1_s = cp.tile([64, 1], F32, tag="g1"); nc.sync.dma_start(g1_s[:], g1c[:])
        b1_s = cp.tile([64, 1], F32, tag="b1"); nc.sync.dma_start(b1_s[:], b1c[:])
        g2_s = cp.tile([128, 2], F32, tag="g2"); nc.sync.dma_start(g2_s[:], g2c[:])
        b2_s = cp.tile([128, 2], F32, tag="b2"); nc.sync.dma_start(b2_s[:], b2c[:])
        g3_s = cp.tile([128, 4], F32, tag="g3"); nc.sync.dma_start(g3_s[:], g3c[:])
        b3_s = cp.tile([128, 4], F32, tag="b3"); nc.sync.dma_start(b3_s[:], b3c[:])
        neghalf = cp.tile([64, 1], F32, tag="neghalf")
        nc.vector.memset(neghalf[:], -0.5)
        onescol = cp.tile([128, 1], F32, tag="onescol")
        nc.vector.memset(onescol[:], 1.0)
        onesrow = cp.tile([1, 512], F32, tag="onesrow")
        nc.vector.memset(onesrow[:], 1.0)


        # persistent per-sample pools
        xb2p = tc.alloc_tile_pool(name="xb2", bufs=S)
        zmt1p = tc.alloc_tile_pool(name="zmt1", bufs=S)
        zmt2p = tc.alloc_tile_pool(name="zmt2", bufs=S)
        zm3p = tc.alloc_tile_pool(name="zm3", bufs=S)
        sacc = tc.alloc_tile_pool(name="sacc", bufs=1)

        xb1p = tc.alloc_tile_pool(name="xb1", bufs=3)
        wk = tc.alloc_tile_pool(name="wk", bufs=1)
        swp = tc.alloc_tile_pool(name="swp", bufs=1)
        afp = tc.alloc_tile_pool(name="afp", bufs=4)
        idxp = tc.alloc_tile_pool(name="idxp", bufs=8)
        gp = tc.alloc_tile_pool(name="gpool", bufs=2)
        usqp = tc.alloc_tile_pool(name="usqp", bufs=2)
        x2p = tc.alloc_tile_pool(name="x2p", bufs=2)
        smallp = tc.alloc_tile_pool(name="small", bufs=4)
        rowp = tc.alloc_tile_pool(name="rowp", bufs=1)
        gvsp = tc.alloc_tile_pool(name="gvsp", bufs=1)

        ps_s = tc.alloc_tile_pool(name="ps_s", bufs=1, space="PSUM")
        ps_uv = tc.alloc_tile_pool(name="ps_uv", bufs=1, space="PSUM")
        ps_gvu = tc.alloc_tile_pool(name="ps_gvu", bufs=2, space="PSUM")
        ps_misc = tc.alloc_tile_pool(name="ps_misc", bufs=1, space="PSUM")
        ps_stat = tc.alloc_tile_pool(name="ps_stat", bufs=1, space="PSUM")

        cr1 = sacc.tile([128, 1], F32, tag="cr1"); nc.vector.memset(cr1[:], 0.0)
        cr2 = sacc.tile([128, 2], F32, tag="cr2"); nc.vector.memset(cr2[:], 0.0)
        s3sq = sacc.tile([128, 4], F32, tag="s3sq"); nc.vector.memset(s3sq[:], 0.0)
        s3s1 = sacc.tile([128, 4], F32, tag="s3s1"); nc.vector.memset(s3s1[:], 0.0)
        acc1 = sacc.tile([2, 256], F32, tag="acc1"); nc.vector.memset(acc1[:], 0.0)
        acc2 = sacc.tile([66, 512], F32, tag="acc2"); nc.vector.memset(acc2[:], 0.0)

        stA = ps_stat.tile([128, 512], F32, tag="stA")
        st3 = ps_stat.tile([128, 4], F32, tag="st3")

        L2G = BF16 if L2_BF16 else F32
        LCFG = {
            1: dict(C=3, Co=64, ncc=1, gdt=F32, zdt=F32, zp=zmt1p, cr=cr1),
            2: dict(C=64, Co=256, ncc=2, gdt=L2G, zdt=L2G, zp=zmt2p, cr=cr2),
        }

        def front(s_i, layer, xb):
            cfg = LCFG[layer]
            C, Co, ncc = cfg["C"], cfg["Co"], cfg["ncc"]
            gdt = cfg["gdt"]
            wn, wd = (w1n_s, w1d_s) if layer == 1 else (w2n_s, w2d_s)
            first = (s_i == 0)
            last = (s_i == S - 1)

            # xx row; XR = [X; -xx/2]
            x2sq = wk.tile([64, 512], F32, tag="x2sq")
            nc.scalar.activation(x2sq[0:C, :], xb[0:C, :], AFT.Square)
            xxp = ps_uv.tile([1, 512], F32, tag="uv")
            nc.tensor.matmul(xxp[:], neghalf[0:C, :], x2sq[0:C, :], start=True, stop=True)
            xr = wk.tile([65, 512], F32, tag="xr")
            nc.scalar.copy(xr[0:C, :], xb[0:C, :])
            xxs = wk.tile([1, 512], F32, tag="xxs")
            nc.scalar.copy(xxs[:], xxp[:])
            nc.sync.dma_start(xr[C:C + 1, :], xxs[:])

            # s-matrix chunks + topk scan
            af = []
            idx24 = []
            for h in range(2):
                sp = ps_s.tile([128, 1024], F32, tag="spsum")
                for q in range(2):
                    mc = h * 2 + q
                    nc.tensor.matmul(
                        sp[:, q * 512:(q + 1) * 512],
                        xb[0:C + 1, mc * 128:(mc + 1) * 128],
                        xr[0:C + 1, :], start=True, stop=True)
                sw2 = swp.tile([128, 1024], F32, tag="sw")
                nc.scalar.copy(sw2[:], sp[:])
                for q in range(2):
                    sw = sw2[:, q * 512:(q + 1) * 512]
                    i24 = idxp.tile([128, 24], U16, tag="idx24")
                    m8 = smallp.tile([128, 24], F32, tag="m8")
                    for r in range(3):
                        nc.vector.max(m8[:, 8 * r:8 * r + 8], sw)
                        nc.vector.max_index(i24[:, 8 * r:8 * r + 8],
                                            m8[:, 8 * r:8 * r + 8], sw)
                        if r < 2:
                            nc.vector.match_replace(sw, m8[:, 8 * r:8 * r + 8], sw, NEG)
                    rep8 = smallp.tile([128, 8], F32, tag="rep8")
                    nc.vector.tensor_copy(rep8[:, 0:4], m8[:, 16:20])
                    nc.vector.memset(rep8[:, 4:8], NEG)
                    nc.vector.match_replace(sw, rep8[:], sw, NEG)
                    a = afp.tile([128, 512], F32, tag="af")
                    nc.vector.tensor_scalar(a[:], sw, NEG, None, AOP.is_equal)
                    af.append(a)
                    idx24.append(i24)

            # int32 index tiles; U^T rows to DRAM; 80 indirect row-gathers
            idx32 = []
            for mc in range(4):
                i32 = idxp.tile([128, 24], mybir.dt.int32, tag="idx32")
                nc.vector.tensor_copy(i32[:, 0:20], idx24[mc][:, 0:20])
                idx32.append(i32)

            usq = usqp.tile([128, 4, 4, Co], F32, tag="usq")
            utd = dramp.tile([N, Co], F32, tag=f"utd{layer}")
            for ic in range(4):
                uvp = ps_uv.tile([128, 2 * Co], F32, tag="uv")
                nc.tensor.matmul(uvp[:, 0:Co], xb[0:C, ic * 128:(ic + 1) * 128],
                                 wn[:], start=True, stop=True)
                nc.tensor.matmul(uvp[:, Co:2 * Co], xb[0:C, ic * 128:(ic + 1) * 128],
                                 wd[:], start=True, stop=True)
                nc.scalar.copy(usq[:, ic, 0:2, :], uvp[:])
                nc.scalar.activation(usq[:, ic, 2:4, :], uvp[:], AFT.Square)
                nc.sync.dma_start(utd[ic * 128:(ic + 1) * 128, :], usq[:, ic, 0, :])

            zmt = cfg["zp"].tile([128, 4, Co], cfg["zdt"], tag=f"zm{layer}")
            for ic in range(4):
                g = gp.tile([128, K, Co], F32, tag="g")
                for q in range(K):
                    nc.gpsimd.indirect_dma_start(
                        out=g[:, q, :], out_offset=None,
                        in_=utd[:],
                        in_offset=bass.IndirectOffsetOnAxis(
                            ap=idx32[ic][:, q:q + 1], axis=0))
                for (na, nb) in ((10, 10), (5, 5), (2, 3), (1, 2), (1, 1)):
                    nc.vector.tensor_tensor(g[:, 0:nb, :], g[:, 0:nb, :],
                                            g[:, na:na + nb, :], AOP.max)
                nc.vector.tensor_tensor(zmt[:, ic, :], g[:, 0, :],
                                        usq[:, ic, 1, :], AOP.add)

            # cross term
            for cc in range(ncc):
                pn = min(128, Co)
                up = ps_gvu.tile([128, 512], F32, tag="gvu")
                nc.tensor.matmul(up[0:pn, :], wn[:, cc * 128:cc * 128 + pn],
                                 xb[0:C, :], start=True, stop=True)
                us = gvsp.tile([128, 512], F32, tag="us")
                nc.scalar.copy(us[0:pn, :], up[0:pn, :])
                gvp = ps_gvu.tile([128, 512], F32, tag="gvu")
                for ic in range(4):
                    nc.tensor.matmul(gvp[0:pn, :],
                                     usq[:, ic, 1, cc * 128:cc * 128 + pn],
                                     af[ic][:], start=(ic == 0), stop=(ic == 3),
                                     skip_group_check=True)
                gvs = gvsp.tile([128, 512], F32, tag="gvs")
                nc.scalar.copy(gvs[0:pn, :], gvp[0:pn, :])
                crx = smallp.tile([128, 1], F32, tag="crx")
                nc.vector.scalar_tensor_tensor(gvs[0:pn, :], us[0:pn, :], 1.0,
                                               gvs[0:pn, :], AOP.mult, AOP.mult,
                                               accum_out=crx[0:pn, :])
                cr = cfg["cr"]
                nc.vector.tensor_tensor(cr[0:pn, cc:cc + 1], cr[0:pn, cc:cc + 1],
                                        crx[0:pn, :], AOP.add)

            # ---- stats ----
            degc = smallp.tile([128, 4], F32, tag="degc")
            for jc in range(4):
                dp = ps_misc.tile([128, 512], F32, tag="misc")
                for ic in range(4):
                    nc.tensor.matmul(dp[:, 0:1], af[ic][:, jc * 128:(jc + 1) * 128],
                                     onescol[:], start=(ic == 0), stop=(ic == 3),
                                     skip_group_check=True)
                nc.scalar.copy(degc[:, jc:jc + 1], dp[:, 0:1])

            # matvec stats into persistent PSUM rows
            for ic in range(4):
                lhs = smallp.tile([128, 2], F32, tag="lhs2")
                nc.vector.tensor_copy(lhs[:, 0:1], degc[:, ic:ic + 1])
                nc.vector.tensor_copy(lhs[:, 1:2], onescol[:])
                if layer == 1:
                    nc.tensor.matmul(
                        stA[0:2, 0:256], lhs[:],
                        usq[:, ic, :, :].rearrange("p a co -> p (a co)"),
                        start=(ic == 0), stop=(ic == 3),
                        skip_group_check=True)
                else:
                    nc.tensor.matmul(
                        stA[32:34, :], lhs[:],
                        usq[:, ic, 0:2, :].rearrange("p a co -> p (a co)"),
                        start=(ic == 0), stop=(ic == 3),
                        skip_group_check=True)
                    nc.tensor.matmul(
                        stA[64:66, :], lhs[:],
                        usq[:, ic, 2:4, :].rearrange("p a co -> p (a co)"),
                        start=(ic == 0), stop=(ic == 3),
                        skip_group_check=True)
            if layer == 1:
                srow = rowp.tile([2, 512], F32, tag="srow")
                nc.scalar.copy(srow[0:2, 0:256], stA[0:2, 0:256])
                nc.vector.tensor_tensor(acc1[:], acc1[:], srow[0:2, 0:256], AOP.add)
            else:
                srow = rowp.tile([66, 512], F32, tag="srow66")
                nc.scalar.copy(srow[32:34, :], stA[32:34, :])
                nc.scalar.copy(srow[64:66, :], stA[64:66, :])
                nc.vector.tensor_tensor(acc2[32:34, :], acc2[32:34, :], srow[32:34, :], AOP.add)
                nc.vector.tensor_tensor(acc2[64:66, :], acc2[64:66, :], srow[64:66, :], AOP.add)

            return zmt

        def stats_finalize(layer, gcol, bcol):
            cfg = LCFG[layer]
            Co, ncc = cfg["Co"], cfg["ncc"]
            pn = min(128, Co)
            if layer == 1:
                sl = dict(s1a=acc1[0:1, 0:64], s1b=acc1[1:2, 64:128],
                          s2a=acc1[0:1, 128:192], s2b=acc1[1:2, 192:256])
            else:
                sl = dict(s1a=acc2[32:33, 0:256], s1b=acc2[33:34, 256:512],
                          s2a=acc2[64:65, 0:256], s2b=acc2[65:66, 256:512])
            nb = 5 * Co
            bin_ = dram1.tile([nb], F32, tag=f"arin{layer}")
            bout = dram1.tile([nb], F32, tag=f"arout{layer}")
            nc.sync.dma_start(bin_[0:Co].rearrange("(o c) -> o c", o=1), sl["s1a"])
            nc.sync.dma_start(bin_[Co:2 * Co].rearrange("(o c) -> o c", o=1), sl["s1b"])
            nc.sync.dma_start(bin_[2 * Co:3 * Co].rearrange("(o c) -> o c", o=1), sl["s2a"])
            nc.sync.dma_start(bin_[3 * Co:4 * Co].rearrange("(o c) -> o c", o=1), sl["s2b"])
            nc.sync.dma_start(
                bin_[4 * Co:5 * Co].rearrange("(cc p) -> p cc", p=pn),
                cfg["cr"][0:pn, 0:ncc])
            nc.gpsimd.collective_compute(
                "AllReduce", AOP.add, replica_groups=[list(range(NC))],
                ins=[bin_.opt()], outs=[bout.opt()])
            r1a = rowp.tile([1, 256], F32, tag="r1a")
            r1b = rowp.tile([1, 256], F32, tag="r1b")
            r2a = rowp.tile([1, 256], F32, tag="r2a")
            r2b = rowp.tile([1, 256], F32, tag="r2b")
            rcr = rowp.tile([1, 256], F32, tag="rcr")
            for t, o in ((r1a, 0), (r1b, 1), (r2a, 2), (r2b, 3), (rcr, 4)):
                nc.sync.dma_start(t[0:1, 0:Co],
                                  bout[o * Co:(o + 1) * Co].rearrange("(o c) -> o c", o=1))
            mean = rowp.tile([1, 256], F32, tag="meanr")
            nc.vector.scalar_tensor_tensor(mean[0:1, 0:Co], r1b[0:1, 0:Co], float(K),
                                           r1a[0:1, 0:Co], AOP.mult, AOP.add)
            nc.vector.tensor_scalar(mean[0:1, 0:Co], mean[0:1, 0:Co], 1.0 / n1,
                                    None, AOP.mult)
            ex2 = rowp.tile([1, 256], F32, tag="ex2r")
            nc.vector.scalar_tensor_tensor(ex2[0:1, 0:Co], rcr[0:1, 0:Co], 2.0,
                                           r2a[0:1, 0:Co], AOP.mult, AOP.add)
            nc.vector.scalar_tensor_tensor(ex2[0:1, 0:Co], r2b[0:1, 0:Co], float(K),
                                           ex2[0:1, 0:Co], AOP.mult, AOP.add)
            nc.vector.tensor_scalar(ex2[0:1, 0:Co], ex2[0:1, 0:Co], 1.0 / n1,
                                    None, AOP.mult)
            var = rowp.tile([1, 256], F32, tag="varr")
            nc.vector.scalar_tensor_tensor(var[0:1, 0:Co], mean[0:1, 0:Co], 1.0,
                                           mean[0:1, 0:Co], AOP.mult, AOP.mult)
            nc.vector.tensor_tensor(var[0:1, 0:Co], ex2[0:1, 0:Co], var[0:1, 0:Co],
                                    AOP.subtract)
            nc.vector.tensor_scalar(var[0:1, 0:Co], var[0:1, 0:Co], EPS, None, AOP.add)
            rec = rowp.tile([1, 256], F32, tag="recr")
            nc.vector.reciprocal(rec[0:1, 0:Co], var[0:1, 0:Co])
            rsq = rowp.tile([1, 256], F32, tag="rsqr")
            nc.scalar.activation(rsq[0:1, 0:Co], rec[0:1, 0:Co], AFT.Sqrt)
            sb = dram1.tile([2 * Co], F32, tag=f"sb{layer}")
            nc.sync.dma_start(sb[0:Co].rearrange("(o c) -> o c", o=1), rsq[0:1, 0:Co])
            nc.sync.dma_start(sb[Co:2 * Co].rearrange("(o c) -> o c", o=1), mean[0:1, 0:Co])
            rsqc = smallp.tile([128, 2], F32, tag=f"rsqc{layer}")
            meanc = smallp.tile([128, 2], F32, tag=f"meanc{layer}")
            nc.sync.dma_start(rsqc[0:pn, 0:ncc],
                              sb[0:Co].rearrange("(cc p) -> p cc", p=pn))
            nc.sync.dma_start(meanc[0:pn, 0:ncc],
                              sb[Co:2 * Co].rearrange("(cc p) -> p cc", p=pn))
            scc = sacc.tile([128, 2], F32, tag=f"scc{layer}")
            shc = sacc.tile([128, 2], F32, tag=f"shc{layer}")
            nc.vector.tensor_tensor(scc[0:pn, 0:ncc], gcol[0:pn, 0:ncc],
                                    rsqc[0:pn, 0:ncc], AOP.mult)
            nc.vector.tensor_tensor(shc[0:pn, 0:ncc], meanc[0:pn, 0:ncc],
                                    scc[0:pn, 0:ncc], AOP.mult)
            nc.vector.tensor_tensor(shc[0:pn, 0:ncc], bcol[0:pn, 0:ncc],
                                    shc[0:pn, 0:ncc], AOP.subtract)
            return scc, shc


        def emit_lrelu(dst, src, sc_ap, sh_ap, pn, accum=None):
            """dst = lrelu(src*sc + sh), slope 0.2; optional accum_out on final op."""
            tt = wk.tile([128, 512], F32, tag="lr_t")
            rr = wk.tile([128, 512], F32, tag="lr_r")
            nc.scalar.activation(tt[0:pn, :], src, AFT.Identity, bias=sh_ap, scale=sc_ap)
            nc.scalar.activation(rr[0:pn, :], src, AFT.Relu, bias=sh_ap, scale=sc_ap)
            nc.vector.scalar_tensor_tensor(rr[0:pn, :], rr[0:pn, :], 4.0, tt[0:pn, :],
                                           AOP.mult, AOP.add)
            if accum is None:
                nc.vector.tensor_scalar(dst, rr[0:pn, :], 0.2, None, AOP.mult)
            else:
                nc.vector.tensor_scalar(dst, rr[0:pn, :], 0.2, 0.0, AOP.mult,
                                        AOP.add, accum_out=accum)

        # ===== phase 1 =====
        zm1 = []
        for s_i in range(S):
            xb = xb1p.tile([4, 512], F32, tag="xb1")
            nc.sync.dma_start(xb[0:3, :], xs[s_i].rearrange("n d -> d n"))
            nc.sync.dma_start(xb[3:4, :], onesrow[:])
            zm1.append(front(s_i, 1, xb))

        if PHASES >= 2:
            sc1, sh1 = stats_finalize(1, g1_s, b1_s)

        # ===== phase 2 =====
        zm2 = []
        xb2s = []
        for s_i in range(S if PHASES >= 3 else 0):
            px = ps_misc.tile([128, 512], F32, tag="misc")
            for ic in range(4):
                nc.tensor.matmul(px[0:64, ic * 128:(ic + 1) * 128], zm1[s_i][:, ic, :],
                                 eye_s[:], is_transpose=True, start=True, stop=True)
            xb2 = xb2p.tile([65, 512], F32, tag="xb2")
            emit_lrelu(xb2[0:64, :], px[0:64, :], sc1[0:64, 0:1], sh1[0:64, 0:1], 64)
            nc.sync.dma_start(xb2[64:65, :], onesrow[:])
            xb2s.append(xb2)
            zm2.append(front(s_i, 2, xb2))

        if PHASES >= 4:
            sc2, sh2 = stats_finalize(2, g2_s, b2_s)

        def build_x2(s_i, cc):
            zdt = LCFG[2]["zdt"]
            idt = eye_b if zdt == BF16 else eye_s
            px = ps_misc.tile([128, 512], zdt, tag="misc")
            for ic in range(4):
                nc.tensor.matmul(px[:, ic * 128:(ic + 1) * 128],
                                 zm2[s_i][:, ic, cc * 128:(cc + 1) * 128],
                                 idt[:], is_transpose=True, start=True, stop=True)
            x2 = x2p.tile([128, 512], F32, tag="x2")
            return px[:], x2

        # ===== phase 3 =====
        zm3s = []
        for s_i in range(S if PHASES >= 5 else 0):
            catsum = smallp.tile([128, 3], F32, tag="catsum")
            x2ab = []
            for cc in range(2):
                px, x2 = build_x2(s_i, cc)
                emit_lrelu(x2[:], px[:], sc2[:, cc:cc + 1], sh2[:, cc:cc + 1], 128,
                           accum=catsum[:, 1 + cc:2 + cc])
                x2ab.append(x2)
            x1scr = wk.tile([64, 512], F32, tag="x1scr")
            nc.scalar.activation(x1scr[:], xb2s[s_i][0:64, :], AFT.Copy,
                                 accum_out=catsum[0:64, 0:1])
            csl = [(catsum[0:64, 0:1], 0), (catsum[0:128, 1:2], 1),
                   (catsum[0:128, 2:3], 2)]
            for cc in range(4):
                for kci, (cs, wi) in enumerate(csl):
                    nc.tensor.matmul(st3[:, cc:cc + 1],
                                     w3ts[wi][:, cc * 128:(cc + 1) * 128], cs,
                                     start=(kci == 0), stop=(kci == 2),
                                     skip_group_check=True)
            st3row = smallp.tile([128, 4], F32, tag="st3row")
            nc.scalar.copy(st3row[:], st3[:])
            nc.vector.tensor_tensor(s3s1[:], s3s1[:], st3row[:], AOP.add)
            zm3 = zm3p.tile([128, 4], F32, tag="zm3")
            rhss = [xb2s[s_i][0:64, :], x2ab[0][:], x2ab[1][:]]
            for cc in range(4):
                zp = ps_s.tile([128, 1024], F32, tag="spsum")
                for kci, rhs in enumerate(rhss):
                    nc.tensor.matmul(zp[:, 0:512],
                                     w3ts[kci][:, cc * 128:(cc + 1) * 128],
                                     rhs,
                                     start=(kci == 0), stop=(kci == 2))
                nc.vector.tensor_reduce(zm3[:, cc:cc + 1], zp[:, 0:512], AXL.X, AOP.max)
                sqs = wk.tile([128, 512], F32, tag="sqscr")
                sqa = smallp.tile([128, 1], F32, tag="sqa")
                nc.scalar.activation(sqs[:], zp[:, 0:512], AFT.Square, accum_out=sqa[:])
                nc.vector.tensor_tensor(s3sq[:, cc:cc + 1], s3sq[:, cc:cc + 1],
                                        sqa[:], AOP.add)
            zm3s.append(zm3)

        # stats3 AR
        if PHASES < 5:
            for pool in (ps_stat, ps_misc, ps_gvu, ps_uv, ps_s, gvsp, rowp, smallp,
                         x2p, usqp, gp, idxp, afp, swp, wk, xb1p, sacc, zm3p, zmt2p,
                         zmt1p, xb2p, dram1, dramp, cp):
                pool.release()
            tcctx.__exit__(None, None, None)
            nc.compile()
            return nc
        s3rows = rowp.tile([128, 8], F32, tag="s3rows")
        nc.vector.tensor_copy(s3rows[:, 0:4], s3s1[:])
        nc.vector.tensor_copy(s3rows[:, 4:8], s3sq[:])
        b3i = dram1.tile([1024], F32, tag="ar3in")
        b3o = dram1.tile([1024], F32, tag="ar3out")
        nc.sync.dma_start(b3i[:].rearrange("(p w) -> p w", p=128), s3rows[:])
        nc.gpsimd.collective_compute("AllReduce", AOP.add,
                                     replica_groups=[list(range(NC))],
                                     ins=[b3i.opt()], outs=[b3o.opt()])
        s3r = rowp.tile([128, 8], F32, tag="s3r")
        nc.sync.dma_start(s3r[:], b3o[:].rearrange("(p w) -> p w", p=128))
        mean3 = sacc.tile([128, 4], F32, tag="mean3")
        nc.vector.tensor_scalar(mean3[:], s3r[:, 0:4], 1.0 / n3, None, AOP.mult)
        var3 = sacc.tile([128, 4], F32, tag="var3")
        nc.vector.tensor_scalar(var3[:], s3r[:, 4:8], 1.0 / n3, None, AOP.mult)
        m3sq = smallp.tile([128, 4], F32, tag="m3sq")
        nc.vector.tensor_tensor(m3sq[:], mean3[:], mean3[:], AOP.mult)
        nc.vector.tensor_tensor(var3[:], var3[:], m3sq[:], AOP.subtract)
        nc.vector.tensor_scalar(var3[:], var3[:], EPS, None, AOP.add)
        rec3 = sacc.tile([128, 4], F32, tag="rec3")
        nc.vector.reciprocal(rec3[:], var3[:])
        sc3 = sacc.tile([128, 4], F32, tag="sc3")
        nc.scalar.activation(sc3[:], rec3[:], AFT.Sqrt)
        nc.vector.tensor_tensor(sc3[:], sc3[:], g3_s[:], AOP.mult)
        sh3 = sacc.tile([128, 4], F32, tag="sh3")
        nc.vector.tensor_tensor(sh3[:], mean3[:], sc3[:], AOP.mult)
        nc.vector.tensor_tensor(sh3[:], b3_s[:], sh3[:], AOP.subtract)

        # ===== phase 4 =====
        for s_i in range(S):
            x2ab = []
            for cc in range(2):
                px, x2 = build_x2(s_i, cc)
                emit_lrelu(x2[:], px[:], sc2[:, cc:cc + 1], sh2[:, cc:cc + 1], 128)
                x2ab.append(x2)
            rhss = [xb2s[s_i][0:64, :], x2ab[0][:], x2ab[1][:]]
            hsum = smallp.tile([128, 4], F32, tag="hsum")
            for cc in range(4):
                zp = ps_s.tile([128, 1024], F32, tag="spsum")
                for kci, rhs in enumerate(rhss):
                    nc.tensor.matmul(zp[:, 0:512],
                                     w3ts[kci][:, cc * 128:(cc + 1) * 128],
                                     rhs,
                                     start=(kci == 0), stop=(kci == 2))
                hscr = wk.tile([128, 512], F32, tag="hscr")
                emit_lrelu(hscr[:], zp[:, 0:512], sc3[:, cc:cc + 1], sh3[:, cc:cc + 1],
                           128, accum=hsum[:, cc:cc + 1])
            # pooled max via commuted bn+lrelu on zm3
            t = smallp.tile([128, 4], F32, tag="tmx")
            nc.vector.tensor_tensor(t[:], zm3s[s_i][:], sc3[:], AOP.mult)
            nc.vector.tensor_tensor(t[:], t[:], sh3[:], AOP.add)
            u = smallp.tile([128, 4], F32, tag="umx")
            nc.vector.tensor_scalar(u[:], t[:], SLOPE, None, AOP.mult)
            hmax = smallp.tile([128, 4], F32, tag="hmax")
            nc.vector.tensor_tensor(hmax[:], t[:], u[:], AOP.max)
            hmean = smallp.tile([128, 4], F32, tag="hmean")
            nc.vector.tensor_scalar(hmean[:], hsum[:], 1.0 / N, None, AOP.mult)
            nc.sync.dma_start(out[s_i, 0:512].rearrange("(cc p) -> p cc", p=128),
                              hmax[:])
            nc.sync.dma_start(out[s_i, 512:1024].rearrange("(cc p) -> p cc", p=128),
                              hmean[:])

        for pool in (ps_stat, ps_misc, ps_gvu, ps_uv, ps_s, gvsp, rowp, smallp,
                     x2p, usqp, gp, idxp, afp, swp, wk, xb1p, sacc, zm3p, zmt2p,
                     zmt1p, xb2p, dram1, dramp, cp):
            pool.release()
    tcctx.__exit__(None, None, None)

    nc.compile()
    return nc


_cached = {}


def kernel(x, w1, g1, b1, w2, g2, b2, w3, g3, b3):
    x = np.asarray(x, dtype=np.float32)
    B, P, Np, D = x.shape
    xs_all = x.reshape(B * P, Np, D)
    w1 = np.asarray(w1, np.float32)
    w2 = np.asarray(w2, np.float32)
    w3 = np.asarray(w3, np.float32)

    if "nc" not in _cached:
        _cached["nc"] = build_nc()
    nc = _cached["nc"]

    common = {
        "wnt1": np.ascontiguousarray(w1[:, 0:3].T),
        "wdt1": np.ascontiguousarray((w1[:, 3:6] - w1[:, 0:3]).T),
        "wnt2": np.ascontiguousarray(w2[:, 0:64].T),
        "wdt2": np.ascontiguousarray((w2[:, 64:128] - w2[:, 0:64]).T),
        "w3t": np.ascontiguousarray(w3.T),
        "g1c": np.asarray(g1, np.float32).reshape(64, 1),
        "b1c": np.asarray(b1, np.float32).reshape(64, 1),
        "g2c": np.ascontiguousarray(np.asarray(g2, np.float32).reshape(2, 128).T),
        "b2c": np.ascontiguousarray(np.asarray(b2, np.float32).reshape(2, 128).T),
        "g3c": np.ascontiguousarray(np.asarray(g3, np.float32).reshape(4, 128).T),
        "b3c": np.ascontiguousarray(np.asarray(b3, np.float32).reshape(4, 128).T),
        "eye": np.eye(128, dtype=np.float32),
    }
    in_maps = []
    for c in range(NC):
        m = dict(common)
        m["xs"] = np.ascontiguousarray(xs_all[c * S:(c + 1) * S])
        in_maps.append(m)

    trace = os.environ.get("KERNEL_TRACE") == "1"
    res = run_bass_kernel_spmd(nc, in_maps, core_ids=list(range(NC)), trace=trace)
    if trace:
        _cached["exec_time_ns"] = res.exec_time_ns
    outs = np.concatenate([res.results[c]["out"] for c in range(NC)], axis=0)
    return outs.reshape(B, P, 1024)
